# revision 20
# baseline (speedup 1.0000x reference)
"""Trainium2 Bass kernel for nn_MultiHeadAttention_9405978378694.

Full-input contract: kernel(**inputs) -> (B, S, DM) float32.

Sharding: tensor-parallel over heads. 16 heads / 8 cores = 2 heads per core.
Each core computes QKV projection for its heads (full sequence, both
batches), causal attention, and a partial out-projection against its slice
of w_out columns. Partials are summed on the host (the all-reduce).

Host-side algebra (exact):
  - The reference applies rotary with frequencies indexed by the HEAD axis
    (not position), so each head's rotation is a constant 128x128 linear map.
    It is folded into the Q/K projection weights: Wq' = R_h @ Wq_h.
  - The V bias enters the output as attn_rows_sum(=1) * b_v, which passes
    linearly through the out-projection: folded into the final bias add as
    w_out @ b_v.
  - Weights are packed host-side so every SBUF partition's data is
    contiguous in HBM (4KB runs instead of 256B scatters).

Device kernel (per core, all matmuls bf16 with fp32 PSUM accumulation):
  QKV in transposed [dh, s] layout -> scores computed DIRECTLY TRANSPOSED
  (lhsT = k-block, rhs = q-tile -> PSUM [t, q]), so no PE transpose of the
  attention matrix is ever needed -> both heads' score blocks packed in one
  two-bank PSUM chunk so a single ScalarE exp covers up to 1024 columns ->
  attn@V with lhsT = expT block and rhs = V augmented with a ones column,
  giving out [q, dh] AND the softmax denominator in column dh of the same
  PSUM tile -> VectorE reciprocal + per-partition tensor_scalar_mul does
  the normalization -> one [q,dh]->[dh,q] PE transpose per (tile, head) ->
  out-projection with both heads in one PSUM group.

Engine balance: ScalarE runs ONLY the exps (it is the softmax throughput
limit at ~1 col/cycle); all PSUM evictions (q/k with bias, V, out-proj
stage) run on VectorE; oT evictions and all output DMAs ride the
otherwise-idle GpSimd queue.

Scheduling notes:
  - per query tile si the PE emission order is scores(si+2), attn@V(si),
    transpose(si-1), out-proj(si-2): the exp chain and the
    reciprocal/normalize/oT-evict chains each get a full iteration of PE
    work to drain, so nothing stalls the PE in steady state;
  - batch 0's trailing transpose/out-proj stages are deferred into batch
    1's QKV section so the pipeline-drain bubble at the batch boundary is
    filled with projection matmuls;
  - startup: whole-tile weight DMAs (one per head per projection) then
    per-kt xt chunks on two alternating HWDGE queues; the kt-outer QKV
    accumulation starts as soon as the four weight tiles + first xt chunk
    land (~1.2MB);
  - batch 1's xt is prefetched on the gpsimd queue during batch 0's
    attention, gated per-tile by WAR on batch 0's QKV reads.
"""

import os
import numpy as np
import ml_dtypes

B, S, DM, H, DH = 2, 2048, 2048, 16, 128
NCORES = 8
HPC = H // NCORES  # heads per core
NT = S // 128      # 128-row tiles along sequence
SCALE = float(DH) ** -0.5

_BUILT = {}
_LAST_IN_MAPS = None


def _build(causal: bool):
    import concourse.mybir as mybir
    import concourse.tile as tile
    from concourse import bacc
    from concourse.masks import make_identity

    f32 = mybir.dt.float32
    bf16 = mybir.dt.bfloat16
    AF = mybir.ActivationFunctionType

    nc = bacc.Bacc("TRN2", target_bir_lowering=False, debug=False)

    xt = nc.dram_tensor("xt", [B, DM, S], bf16, kind="ExternalInput")
    # partition-contiguous packed weights: [p, kt, m] = w[kt*128+p, m]
    wq = nc.dram_tensor("wq", [HPC, 128, 16 * DH], bf16, kind="ExternalInput")
    wk = nc.dram_tensor("wk", [HPC, 128, 16 * DH], bf16, kind="ExternalInput")
    wv = nc.dram_tensor("wv", [128, 16 * HPC * DH], bf16, kind="ExternalInput")
    bq = nc.dram_tensor("bq", [HPC, DH], f32, kind="ExternalInput")
    bk = nc.dram_tensor("bk", [HPC, DH], f32, kind="ExternalInput")
    wo = nc.dram_tensor("wo", [HPC, DH, DM], bf16, kind="ExternalInput")
    if not causal:
        # host sends the mask TRANSPOSED ([t, q] orientation)
        mb = nc.dram_tensor("maskb", [S, S], f32, kind="ExternalInput")
    outp = nc.dram_tensor("outp", [B, S, DM], f32, kind="ExternalOutput")

    from contextlib import ExitStack
    with tile.TileContext(nc) as tc:
        with ExitStack() as es:
            constp = es.enter_context(tc.tile_pool(name="const", bufs=1))
            wqkp = es.enter_context(tc.tile_pool(name="wqk", bufs=1))
            wvp = es.enter_context(tc.tile_pool(name="wvp", bufs=1))
            wop = es.enter_context(tc.tile_pool(name="wop", bufs=1))
            xtp = es.enter_context(tc.tile_pool(name="xtp", bufs=1))
            qkvp = es.enter_context(tc.tile_pool(name="qkv", bufs=1))
            expp = es.enter_context(tc.tile_pool(name="expT", bufs=3))
            accp = es.enter_context(tc.tile_pool(name="accs", bufs=4))
            osbp = es.enter_context(tc.tile_pool(name="osb", bufs=2))
            outTp = es.enter_context(tc.tile_pool(name="outT", bufs=2))
            outsp = es.enter_context(tc.tile_pool(name="ostage", bufs=2))
            mbp = es.enter_context(tc.tile_pool(name="mbp", bufs=3))
            # PSUM budget (8 banks of 2KB/partition):
            #   A   3 x [128,512]f32   = 3 banks (out-proj, transposes, warmup)
            #   psT 2 x [128,1024]f32  = 4 banks (scores chunks, q|k packed
            #                                     per head, V-proj groups)
            #   po  1 x [128,2,132]f32 = 1 bank  (attn@V out + softmax denom)
            Ap = es.enter_context(tc.tile_pool(name="A", bufs=3, space="PSUM"))
            psTp = es.enter_context(tc.tile_pool(name="psT", bufs=2, space="PSUM"))
            pop = es.enter_context(tc.tile_pool(name="po", bufs=1, space="PSUM"))

            ident = constp.tile([128, 128], bf16)
            make_identity(nc, ident[:])
            warm_src = constp.tile([128, 512], bf16)
            nc.gpsimd.memset(warm_src[:], 0.5)
            # causal 0/1 triangle for the transposed-diagonal block: the
            # diagonal is masked AFTER exp by zeroing eT on the gpsimd
            # engine (SBUF-only; gpsimd cannot touch PSUM), which keeps
            # the score->exp chain free of extra PSUM ops
            tri01 = constp.tile([128, 128], bf16)
            nc.gpsimd.memset(tri01[:], 1.0)
            nc.gpsimd.affine_select(
                out=tri01[:], in_=tri01[:],
                compare_op=mybir.AluOpType.is_ge, fill=0.0,
                base=0, pattern=[[1, 128]], channel_multiplier=-1,
            )

            wq_sb = [wqkp.tile([128, 16, DH], bf16, tag=f"wq{h}", name=f"wq{h}")
                     for h in range(HPC)]
            wk_sb = [wqkp.tile([128, 16, DH], bf16, tag=f"wk{h}", name=f"wk{h}")
                     for h in range(HPC)]
            wv_sb = wvp.tile([128, 16, HPC * DH], bf16, tag="wv", name="wv")
            wo_t = [wop.tile([128, DM], bf16, tag=f"wo{h}", name=f"wo{h}") for h in range(HPC)]
            bq_t = [constp.tile([128, 1], f32, tag=f"bq{h}", name=f"bq{h}") for h in range(HPC)]
            bk_t = [constp.tile([128, 1], f32, tag=f"bk{h}", name=f"bk{h}") for h in range(HPC)]

            # V for both heads with a ones column per (j, h):
            # [t_local, j, h, 0:128]=V, [t_local, j, h, 128]=1.0 (softmax denom)
            v_sb = qkvp.tile([128, NT, HPC, DH + 1], bf16, tag="v", name="v_aug")
            nc.gpsimd.memset(v_sb[:, :, :, DH:DH + 1], 1.0)

            # dummy matmuls bridge the PE to the first weight/xt arrivals so
            # the p-state is ramped when real work starts
            warm_ps = Ap.tile([128, 512], f32, tag="A", name="warm_ps")
            for _ in range(28):
                nc.tensor.matmul(warm_ps[:], lhsT=ident[:], rhs=warm_src[:],
                                 start=True, stop=True)
            for h in range(HPC):
                nc.gpsimd.dma_start(out=bq_t[h][:],
                                    in_=bq.ap()[h].rearrange("(d o) -> d o", o=1))
                nc.gpsimd.dma_start(out=bk_t[h][:],
                                    in_=bk.ap()[h].rearrange("(d o) -> d o", o=1))

            xts0 = [xtp.tile([128, S], bf16, tag=f"xt{kt}", name=f"xt_0_{kt}")
                    for kt in range(16)]
            # startup across all three HWDGE queues (~140GB/s each):
            # sync/scalar carry the weights + the sc0/sc1 xt chunks the
            # kt-outer and first sc-outer loops need; gpsimd carries the
            # sc2+sc3 xt halves needed last
            nc.sync.dma_start(out=wq_sb[0][:], in_=wq.ap()[0])
            nc.scalar.dma_start(out=wk_sb[0][:], in_=wk.ap()[0])
            nc.sync.dma_start(out=wq_sb[1][:], in_=wq.ap()[1])
            nc.scalar.dma_start(out=wk_sb[1][:], in_=wk.ap()[1])
            nc.scalar.dma_start(out=wv_sb[:], in_=wv.ap()[:])
            for kt in range(16):
                eng = nc.sync if kt % 2 == 0 else nc.scalar
                eng.dma_start(
                    out=xts0[kt][:, 0:512],
                    in_=xt.ap()[0, kt * 128:(kt + 1) * 128, 0:512])
            for kt in range(16):
                eng = nc.sync if kt % 2 == 0 else nc.scalar
                eng.dma_start(
                    out=xts0[kt][:, 512:1024],
                    in_=xt.ap()[0, kt * 128:(kt + 1) * 128, 512:1024])
            for kt in range(16):
                nc.gpsimd.dma_start(
                    out=xts0[kt][:, 1024:2048],
                    in_=xt.ap()[0, kt * 128:(kt + 1) * 128, 1024:2048])
            nc.sync.dma_start(out=wo_t[0][:], in_=wo.ap()[0])
            nc.scalar.dma_start(out=wo_t[1][:], in_=wo.ap()[1])

            pending_tail = []  # deferred trailing stages of the previous batch

            for b in range(B):
                if b == 0:
                    xts = xts0
                else:
                    # prefetched on the gpsimd queue during batch 0's
                    # attention; WAR deps gate each tile on batch 0's QKV
                    xts = [xtp.tile([128, S], bf16, tag=f"xt{kt}",
                                    name=f"xt_1_{kt}")
                           for kt in range(16)]
                    for kt in range(16):
                        nc.gpsimd.dma_start(
                            out=xts[kt][:],
                            in_=xt.ap()[b, kt * 128:(kt + 1) * 128, :])

                q_sb = [qkvp.tile([128, S], bf16, tag=f"q{h}", name=f"q_{b}_{h}") for h in range(HPC)]
                k_sb = [qkvp.tile([128, S], bf16, tag=f"k{h}", name=f"k_{b}_{h}") for h in range(HPC)]

                # ---- QKV projection ----
                # first s-chunk: q/k kt-outer, consuming each xt piece as it
                # arrives from HBM instead of waiting for all 16.  Each head
                # packs its q (bank A) and k (bank B) groups in one psT tile.
                ssl = slice(0, 512)
                psqk = [psTp.tile([128, 1024], f32, tag="psT",
                                  name=f"psqk0_{b}_{h}") for h in range(HPC)]
                for kt in range(16):
                    for h in range(HPC):
                        nc.tensor.matmul(
                            psqk[h][:, 0:512], lhsT=wq_sb[h][:, kt, :],
                            rhs=xts[kt][:, ssl],
                            start=(kt == 0), stop=(kt == 15))
                        nc.tensor.matmul(
                            psqk[h][:, 512:1024], lhsT=wk_sb[h][:, kt, :],
                            rhs=xts[kt][:, ssl],
                            start=(kt == 0), stop=(kt == 15))
                for h in range(HPC):
                    nc.vector.tensor_scalar_add(q_sb[h][:, ssl],
                                                psqk[h][:, 0:512], bq_t[h][:])
                    nc.vector.tensor_scalar_add(k_sb[h][:, ssl],
                                                psqk[h][:, 512:1024], bk_t[h][:])
                # V st-outer (xt sc0 has fully landed by now); the four
                # 256-wide groups in one two-bank tile run sequentially —
                # two concurrently-open accumulation groups in one PSUM
                # bank corrupt the first term
                psv = psTp.tile([128, 1024], f32, tag="psT",
                                name=f"psv0_{b}")
                for st in range(4):
                    vsl = slice(st * 256, st * 256 + HPC * DH)
                    for kt in range(16):
                        nc.tensor.matmul(
                            psv[:, vsl],
                            lhsT=xts[kt][:, st * 128:(st + 1) * 128],
                            rhs=wv_sb[:, kt, :],
                            start=(kt == 0), stop=(kt == 15))
                    nc.vector.tensor_copy(v_sb[:, st, :, 0:DH], psv[:, vsl])
                # previous batch's trailing transpose/out-proj: the PSUM
                # banks are free again here, and these matmuls fill the
                # batch-boundary pipeline drain
                for fn in pending_tail:
                    fn()
                pending_tail = []
                for sc4 in range(1, 4):
                    ssl = slice(sc4 * 512, (sc4 + 1) * 512)
                    for h in range(HPC):
                        ps = psTp.tile([128, 1024], f32, tag="psT")
                        for kt in range(16):
                            nc.tensor.matmul(
                                ps[:, 0:512], lhsT=wq_sb[h][:, kt, :],
                                rhs=xts[kt][:, ssl],
                                start=(kt == 0), stop=(kt == 15))
                            nc.tensor.matmul(
                                ps[:, 512:1024], lhsT=wk_sb[h][:, kt, :],
                                rhs=xts[kt][:, ssl],
                                start=(kt == 0), stop=(kt == 15))
                        nc.vector.tensor_scalar_add(q_sb[h][:, ssl],
                                                    ps[:, 0:512], bq_t[h][:])
                        nc.vector.tensor_scalar_add(k_sb[h][:, ssl],
                                                    ps[:, 512:1024], bk_t[h][:])
                    for st4 in range(4):
                        st = sc4 * 4 + st4
                        tsl = slice(st * 128, (st + 1) * 128)
                        psv = psTp.tile([128, 1024], f32, tag="psT")
                        for kt in range(16):
                            nc.tensor.matmul(
                                psv[:, :HPC * DH], lhsT=xts[kt][:, tsl],
                                rhs=wv_sb[:, kt, :],
                                start=(kt == 0), stop=(kt == 15))
                        nc.vector.tensor_copy(v_sb[:, st, :, 0:DH],
                                              psv[:, :HPC * DH])

                # ---- attention ----
                def stage_scores(si, b=b, q_sb=q_sb, k_sb=k_sb):
                    """Transposed scores + exp for both heads of q-tile si.

                    Each two-bank PSUM chunk holds up to 4 j-blocks x 2
                    heads in [j, h, q] column order, so one ScalarE exp
                    covers the whole chunk.  Only the diagonal block gets
                    the causal mask; staircase blocks beyond the diagonal
                    are never consumed."""
                    nj = si + 1 if causal else NT
                    eT = expp.tile([128, nj, HPC, 128], bf16, tag="expT",
                                   name=f"expT_{b}_{si}")
                    for c4 in range(0, nj, 4):
                        jn = min(4, nj - c4)
                        psT = psTp.tile([128, 4, HPC, 128], f32, tag="psT",
                                        name=f"psT_{b}_{si}_{c4}")
                        for j4 in range(jn):
                            j = c4 + j4
                            for h in range(HPC):
                                nc.tensor.matmul(
                                    psT[:, j4, h, :],
                                    lhsT=k_sb[h][:, j * 128:(j + 1) * 128],
                                    rhs=q_sb[h][:, si * 128:(si + 1) * 128],
                                    start=True, stop=True)
                        if not causal:
                            mt = mbp.tile([128, 512], f32, tag="mb",
                                          name=f"mb_{b}_{si}_{c4}")
                            nc.sync.dma_start(
                                out=mt[:, :jn * 128],
                                in_=mb.ap()[c4 * 128:(c4 + jn) * 128,
                                            si * 128:(si + 1) * 128]
                                .rearrange("(j t) q -> t (j q)", t=128))
                            for h in range(HPC):
                                nc.vector.tensor_add(
                                    psT[:, :jn, h, :], psT[:, :jn, h, :],
                                    mt[:, :jn * 128])
                        nc.scalar.activation(
                            eT[:, c4:c4 + jn, :, :],
                            psT[:, :jn, :, :], AF.Exp, scale=SCALE)
                    if causal:
                        # zero the exp'd upper triangle of the diagonal block
                        # (identical to masking scores with -inf before exp)
                        for h in range(HPC):
                            nc.gpsimd.tensor_mul(eT[:, nj - 1, h, :],
                                                 eT[:, nj - 1, h, :], tri01[:])
                    return eT

                def stage_attnv(si, eT, b=b):
                    """attn@V with the ones-augmented V: PSUM [q, DH+1] where
                    column DH is the softmax denominator. Normalization =
                    VectorE reciprocal + per-partition tensor_scalar_mul."""
                    nj = si + 1 if causal else NT
                    po = pop.tile([128, HPC, DH + 4], f32, tag="po",
                                  name=f"po_{b}_{si}")  # [:, h, DH] = denom
                    o_h = []
                    for h in range(HPC):
                        for j in range(nj):
                            nc.tensor.matmul(
                                po[:, h, 0:DH + 1],
                                lhsT=eT[:, j, h, :],
                                rhs=v_sb[:, j, h, :],
                                start=(j == 0), stop=(j == nj - 1))
                        rinv = accp.tile([128, 1], f32, tag=f"rinv{h}",
                                         name=f"rinv_{b}_{si}_{h}")
                        nc.vector.reciprocal(rinv[:], po[:, h, DH:DH + 1])
                        o_sb = osbp.tile([128, DH], bf16, tag=f"osb{h}",
                                         name=f"osb_{b}_{si}_{h}")
                        nc.vector.tensor_scalar_mul(o_sb[:], po[:, h, 0:DH],
                                                    rinv[:])
                        o_h.append(o_sb)
                    return o_h

                def stage_transp(si, o_h, b=b):
                    """[q,dh]->[dh,q] PE transpose per head; bf16 eviction on
                    the gpsimd engine (scalar and vector are both loaded)."""
                    ptr = Ap.tile([128, 512], f32, tag="A",
                                  name=f"ptr_{b}_{si}")
                    for h in range(HPC):
                        nc.tensor.matmul(
                            ptr[:, h * DH:(h + 1) * DH],
                            lhsT=o_h[h][:], rhs=ident[:],
                            start=True, stop=True)
                    oT = outTp.tile([128, HPC * DH], bf16, tag="outT",
                                    name=f"oT_{b}_{si}")
                    nc.vector.tensor_copy(oT[:], ptr[:, :HPC * DH])
                    return oT

                def stage_oproj(si, oT, b=b):
                    """Out-projection with both heads in one PSUM group."""
                    ostage = outsp.tile([128, DM], f32, tag="ostage",
                                        name=f"ostage_{b}_{si}")
                    for ncn in range(4):
                        nsl = slice(ncn * 512, (ncn + 1) * 512)
                        pso = Ap.tile([128, 512], f32, tag="A",
                                      name=f"pso_{b}_{si}_{ncn}")
                        for h in range(HPC):
                            nc.tensor.matmul(pso[:],
                                             lhsT=oT[:, h * DH:(h + 1) * DH],
                                             rhs=wo_t[h][:, nsl],
                                             start=(h == 0), stop=(h == HPC - 1))
                        if ncn % 2 == 0:
                            nc.scalar.activation(ostage[:, nsl], pso[:],
                                                 AF.Copy)
                        else:
                            nc.vector.tensor_copy(ostage[:, nsl], pso[:])
                        last_tile = (si == NT - 1)
                        if last_tile:
                            # fine-grained stores so the tail drains early
                            nc.sync.dma_start(
                                out=outp.ap()[b, si * 128:(si + 1) * 128, nsl],
                                in_=ostage[:, nsl])
                        elif ncn % 2 == 1:
                            nc.sync.dma_start(
                                out=outp.ap()[b, si * 128:(si + 1) * 128,
                                              (ncn - 1) * 512:(ncn + 1) * 512],
                                in_=ostage[:, (ncn - 1) * 512:(ncn + 1) * 512])

                # software pipeline: scores two tiles ahead; transpose one
                # and out-proj two tiles behind attn@V
                exp_q = {0: stage_scores(0), 1: stage_scores(1)}
                o_box, oT_box = {}, {}
                for si in range(NT):
                    if si + 2 < NT:
                        exp_q[si + 2] = stage_scores(si + 2)
                    o_box[si] = stage_attnv(si, exp_q.pop(si))
                    if si >= 1:
                        oT_box[si - 1] = stage_transp(si - 1, o_box.pop(si - 1))
                    if si >= 2:
                        stage_oproj(si - 2, oT_box.pop(si - 2))
                pending_tail = [
                    (lambda si=NT - 1, oh=o_box.pop(NT - 1):
                     oT_box.__setitem__(si, stage_transp(si, oh))),
                    (lambda si=NT - 2: stage_oproj(si, oT_box.pop(si))),
                    (lambda si=NT - 1: stage_oproj(si, oT_box.pop(si))),
                ]

            # final batch's trailing stages
            for fn in pending_tail:
                fn()

    nc.compile()
    return nc


def _get(causal: bool):
    if causal not in _BUILT:
        _BUILT[causal] = _build(causal)
    return _BUILT[causal]


def _rot(fr, fi, m):
    """Apply the reference's per-head rotary as a linear map on rows of m."""
    top, bot = m[:DH // 2], m[DH // 2:]
    return np.concatenate([fr[:, None] * top - fi[:, None] * bot,
                           fi[:, None] * top + fr[:, None] * bot], axis=0)


def _pack(w):
    """[DM, C] -> [128, 16*C] with [p, kt*C+c] = w[kt*128+p, c]."""
    c = w.shape[1]
    return np.ascontiguousarray(
        w.reshape(16, 128, c).transpose(1, 0, 2).reshape(128, 16 * c))


def kernel(x, w_qkv, b_qkv, w_out, b_out, fc_real, fc_imag, mask):
    x = np.asarray(x, np.float32)
    w_qkv = np.asarray(w_qkv, np.float32)
    b_qkv = np.asarray(b_qkv, np.float32)
    w_out = np.asarray(w_out, np.float32)
    b_out = np.asarray(b_out, np.float32)
    fc_real = np.asarray(fc_real, np.float32)
    fc_imag = np.asarray(fc_imag, np.float32)
    mask_np = np.asarray(mask)[0, 0]

    causal = bool(np.array_equal(
        mask_np, np.triu(np.ones((S, S), bool), 1)))

    bf = ml_dtypes.bfloat16
    xt_host = np.ascontiguousarray(x.transpose(0, 2, 1)).astype(bf)

    in_maps = []
    maskb = None
    if not causal:
        # transposed ([t, q]) additive mask for the transposed-scores layout
        maskb = np.ascontiguousarray(
            np.where(mask_np, np.float32(-1e30), np.float32(0.0)).T)
    for c in range(NCORES):
        wq_h, wk_h, bq_h, bk_h, wv_h, wo_h = [], [], [], [], [], []
        for hh in range(HPC):
            g = c * HPC + hh
            fr = fc_real[0, g, :]
            fi = fc_imag[0, g, :]
            wq_h.append(_pack(
                _rot(fr, fi, w_qkv[g * DH:(g + 1) * DH, :]).T).astype(bf))
            wk_h.append(_pack(
                _rot(fr, fi, w_qkv[DM + g * DH:DM + (g + 1) * DH, :]).T
            ).astype(bf))
            bq_h.append(_rot(fr, fi, b_qkv[g * DH:(g + 1) * DH, None])[:, 0])
            bk_h.append(_rot(fr, fi,
                             b_qkv[DM + g * DH:DM + (g + 1) * DH, None])[:, 0])
            wv_h.append(w_qkv[2 * DM + g * DH:2 * DM + (g + 1) * DH, :].T)
            wo_h.append(np.ascontiguousarray(
                w_out[:, g * DH:(g + 1) * DH].T).astype(bf))
        m = {
            "xt": xt_host,
            "wq": np.stack(wq_h),
            "wk": np.stack(wk_h),
            "wv": _pack(np.concatenate(wv_h, axis=1)).astype(bf),
            "bq": np.stack(bq_h).astype(np.float32),
            "bk": np.stack(bk_h).astype(np.float32),
            "wo": np.stack(wo_h),
        }
        if not causal:
            m["maskb"] = maskb
        in_maps.append(m)

    from concourse.bass_utils import run_bass_kernel_spmd
    nc = _get(causal)
    global _LAST_IN_MAPS
    _LAST_IN_MAPS = in_maps
    trace = os.environ.get("MHA_TRACE") == "1"
    res = run_bass_kernel_spmd(nc, in_maps, core_ids=list(range(NCORES)),
                               trace=trace)
    if trace:
        kernel.last_results = res

    out = res.results[0]["outp"].astype(np.float64)
    for c in range(1, NCORES):
        out += res.results[c]["outp"]
    b_v = b_qkv[2 * DM:]
    out += (b_out + w_out @ b_v)[None, None, :]
    return out.astype(np.float32)


# revision 21
# speedup vs baseline: 1.1416x; 1.1416x over previous
"""Trainium2 Bass kernel for nn_MultiHeadAttention_9405978378694.

Full-input contract: kernel(**inputs) -> (B, S, DM) float32.

Sharding: tensor-parallel over heads. 16 heads / 8 cores = 2 heads per core.
Each core computes QKV projection for its heads (full sequence, both
batches), causal attention, and a partial out-projection against its slice
of w_out columns. Partials are summed on the host (the all-reduce).

Host-side algebra (exact):
  - The reference applies rotary with frequencies indexed by the HEAD axis
    (not position), so each head's rotation is a constant 128x128 linear map.
    It is folded into the Q/K projection weights: Wq' = R_h @ Wq_h.
  - The V bias enters the output as attn_rows_sum(=1) * b_v, which passes
    linearly through the out-projection: folded into the final bias add as
    w_out @ b_v.
  - Weights are packed host-side so every SBUF partition's data is
    contiguous in HBM (4KB runs instead of 256B scatters).

Device kernel (per core, all matmuls bf16 with fp32 PSUM accumulation):
  QKV in transposed [dh, s] layout -> scores computed DIRECTLY TRANSPOSED
  (lhsT = k-block, rhs = q-tile -> PSUM [t, q]), so no PE transpose of the
  attention matrix is ever needed -> both heads' score blocks packed in one
  two-bank PSUM chunk so a single ScalarE exp covers up to 1024 columns ->
  attn@V with lhsT = expT block and rhs = V augmented with a ones column,
  giving out [q, dh] AND the softmax denominator in column dh of the same
  PSUM tile -> VectorE reciprocal + per-partition tensor_scalar_mul does
  the normalization -> one [q,dh]->[dh,q] PE transpose per (tile, head) ->
  out-projection with both heads in one PSUM group.

Engine balance: ScalarE runs ONLY the exps (it is the softmax throughput
limit at ~1 col/cycle); all PSUM evictions (q/k with bias, V, out-proj
stage) run on VectorE; oT evictions and all output DMAs ride the
otherwise-idle GpSimd queue.

Scheduling notes:
  - per query tile si the PE emission order is scores(si+2), attn@V(si),
    transpose(si-1), out-proj(si-2): the exp chain and the
    reciprocal/normalize/oT-evict chains each get a full iteration of PE
    work to drain, so nothing stalls the PE in steady state;
  - batch 0's trailing transpose/out-proj stages are deferred into batch
    1's QKV section so the pipeline-drain bubble at the batch boundary is
    filled with projection matmuls;
  - startup: whole-tile weight DMAs (one per head per projection) then
    per-kt xt chunks on two alternating HWDGE queues; the kt-outer QKV
    accumulation starts as soon as the four weight tiles + first xt chunk
    land (~1.2MB);
  - batch 1's xt is prefetched on the gpsimd queue during batch 0's
    attention, gated per-tile by WAR on batch 0's QKV reads.
"""

import os
import numpy as np
import ml_dtypes

B, S, DM, H, DH = 2, 2048, 2048, 16, 128
NCORES = 8
HPC = H // NCORES  # heads per core
NT = S // 128      # 128-row tiles along sequence
SCALE = float(DH) ** -0.5

_BUILT = {}
_LAST_IN_MAPS = None


def _build(causal: bool):
    import concourse.mybir as mybir
    import concourse.tile as tile
    from concourse import bacc
    from concourse.masks import make_identity

    f32 = mybir.dt.float32
    bf16 = mybir.dt.bfloat16
    AF = mybir.ActivationFunctionType

    nc = bacc.Bacc("TRN2", target_bir_lowering=False, debug=False)

    xt = nc.dram_tensor("xt", [B, DM, S], bf16, kind="ExternalInput")
    # partition-contiguous packed weights: [p, kt, m] = w[kt*128+p, m]
    wq = nc.dram_tensor("wq", [HPC, 128, 16 * DH], bf16, kind="ExternalInput")
    wk = nc.dram_tensor("wk", [HPC, 128, 16 * DH], bf16, kind="ExternalInput")
    wv = nc.dram_tensor("wv", [128, 16 * HPC * DH], bf16, kind="ExternalInput")
    bq = nc.dram_tensor("bq", [HPC, DH], f32, kind="ExternalInput")
    bk = nc.dram_tensor("bk", [HPC, DH], f32, kind="ExternalInput")
    wo = nc.dram_tensor("wo", [HPC, DH, DM], bf16, kind="ExternalInput")
    if not causal:
        # host sends the mask TRANSPOSED ([t, q] orientation)
        mb = nc.dram_tensor("maskb", [S, S], f32, kind="ExternalInput")
    outp = nc.dram_tensor("outp", [B, S, DM], f32, kind="ExternalOutput")

    from contextlib import ExitStack
    with tile.TileContext(nc) as tc:
        with ExitStack() as es:
            constp = es.enter_context(tc.tile_pool(name="const", bufs=1))
            wqkp = es.enter_context(tc.tile_pool(name="wqk", bufs=1))
            wvp = es.enter_context(tc.tile_pool(name="wvp", bufs=1))
            wop = es.enter_context(tc.tile_pool(name="wop", bufs=1))
            xtp = es.enter_context(tc.tile_pool(name="xtp", bufs=1))
            qkvp = es.enter_context(tc.tile_pool(name="qkv", bufs=1))
            expp = es.enter_context(tc.tile_pool(name="expT", bufs=3))
            accp = es.enter_context(tc.tile_pool(name="accs", bufs=4))
            osbp = es.enter_context(tc.tile_pool(name="osb", bufs=2))
            outTp = es.enter_context(tc.tile_pool(name="outT", bufs=2))
            outsp = es.enter_context(tc.tile_pool(name="ostage", bufs=2))
            mbp = es.enter_context(tc.tile_pool(name="mbp", bufs=3))
            # PSUM budget (8 banks of 2KB/partition):
            #   A   2 x [128,512]f32   = 2 banks (out-proj, transposes, warmup)
            #   psT 2 x [128,1024]f32  = 4 banks (scores chunks, q|k packed
            #                                     per head, V-proj groups)
            #   po  2 x [128,2,132]f32 = 2 banks (attn@V out + softmax denom)
            Ap = es.enter_context(tc.tile_pool(name="A", bufs=2, space="PSUM"))
            psTp = es.enter_context(tc.tile_pool(name="psT", bufs=2, space="PSUM"))
            pop = es.enter_context(tc.tile_pool(name="po", bufs=2, space="PSUM"))

            ident = constp.tile([128, 128], bf16)
            make_identity(nc, ident[:])
            warm_src = constp.tile([128, 512], bf16)
            nc.gpsimd.memset(warm_src[:], 0.5)
            # causal 0/1 triangle for the transposed-diagonal block: the
            # diagonal is masked AFTER exp by zeroing eT on the gpsimd
            # engine (SBUF-only; gpsimd cannot touch PSUM), which keeps
            # the score->exp chain free of extra PSUM ops
            tri01 = constp.tile([128, 128], bf16)
            nc.gpsimd.memset(tri01[:], 1.0)
            nc.gpsimd.affine_select(
                out=tri01[:], in_=tri01[:],
                compare_op=mybir.AluOpType.is_ge, fill=0.0,
                base=0, pattern=[[1, 128]], channel_multiplier=-1,
            )

            wq_sb = [wqkp.tile([128, 16, DH], bf16, tag=f"wq{h}", name=f"wq{h}")
                     for h in range(HPC)]
            wk_sb = [wqkp.tile([128, 16, DH], bf16, tag=f"wk{h}", name=f"wk{h}")
                     for h in range(HPC)]
            wv_sb = wvp.tile([128, 16, HPC * DH], bf16, tag="wv", name="wv")
            wo_t = [wop.tile([128, DM], bf16, tag=f"wo{h}", name=f"wo{h}") for h in range(HPC)]
            bq_t = [constp.tile([128, 1], f32, tag=f"bq{h}", name=f"bq{h}") for h in range(HPC)]
            bk_t = [constp.tile([128, 1], f32, tag=f"bk{h}", name=f"bk{h}") for h in range(HPC)]

            # V for both heads with a ones column per (j, h):
            # [t_local, j, h, 0:128]=V, [t_local, j, h, 128]=1.0 (softmax denom)
            v_sb = qkvp.tile([128, NT, HPC, DH + 1], bf16, tag="v", name="v_aug")
            nc.gpsimd.memset(v_sb[:, :, :, DH:DH + 1], 1.0)

            # dummy matmuls bridge the PE to the first weight/xt arrivals so
            # the p-state is ramped when real work starts
            warm_ps = Ap.tile([128, 512], f32, tag="A", name="warm_ps")
            for _ in range(28):
                nc.tensor.matmul(warm_ps[:], lhsT=ident[:], rhs=warm_src[:],
                                 start=True, stop=True)
            for h in range(HPC):
                nc.gpsimd.dma_start(out=bq_t[h][:],
                                    in_=bq.ap()[h].rearrange("(d o) -> d o", o=1))
                nc.gpsimd.dma_start(out=bk_t[h][:],
                                    in_=bk.ap()[h].rearrange("(d o) -> d o", o=1))

            xts0 = [xtp.tile([128, S], bf16, tag=f"xt{kt}", name=f"xt_0_{kt}")
                    for kt in range(16)]
            # startup across all three HWDGE queues (~140GB/s each):
            # sync/scalar carry the weights + the sc0/sc1 xt chunks the
            # kt-outer and first sc-outer loops need; gpsimd carries the
            # sc2+sc3 xt halves needed last
            nc.sync.dma_start(out=wq_sb[0][:], in_=wq.ap()[0])
            nc.scalar.dma_start(out=wk_sb[0][:], in_=wk.ap()[0])
            nc.sync.dma_start(out=wq_sb[1][:], in_=wq.ap()[1])
            nc.scalar.dma_start(out=wk_sb[1][:], in_=wk.ap()[1])
            nc.scalar.dma_start(out=wv_sb[:], in_=wv.ap()[:])
            for kt in range(16):
                eng = nc.sync if kt % 2 == 0 else nc.scalar
                eng.dma_start(
                    out=xts0[kt][:, 0:512],
                    in_=xt.ap()[0, kt * 128:(kt + 1) * 128, 0:512])
            for kt in range(16):
                eng = nc.sync if kt % 2 == 0 else nc.scalar
                eng.dma_start(
                    out=xts0[kt][:, 512:1024],
                    in_=xt.ap()[0, kt * 128:(kt + 1) * 128, 512:1024])
            for kt in range(16):
                nc.gpsimd.dma_start(
                    out=xts0[kt][:, 1024:2048],
                    in_=xt.ap()[0, kt * 128:(kt + 1) * 128, 1024:2048])
            nc.sync.dma_start(out=wo_t[0][:], in_=wo.ap()[0])
            nc.scalar.dma_start(out=wo_t[1][:], in_=wo.ap()[1])

            pending_tail = []  # deferred trailing stages of the previous batch

            for b in range(B):
                if b == 0:
                    xts = xts0
                else:
                    # prefetched on the gpsimd queue during batch 0's
                    # attention; WAR deps gate each tile on batch 0's QKV
                    xts = [xtp.tile([128, S], bf16, tag=f"xt{kt}",
                                    name=f"xt_1_{kt}")
                           for kt in range(16)]
                    for kt in range(16):
                        nc.gpsimd.dma_start(
                            out=xts[kt][:],
                            in_=xt.ap()[b, kt * 128:(kt + 1) * 128, :])

                q_sb = [qkvp.tile([128, S], bf16, tag=f"q{h}", name=f"q_{b}_{h}") for h in range(HPC)]
                k_sb = [qkvp.tile([128, S], bf16, tag=f"k{h}", name=f"k_{b}_{h}") for h in range(HPC)]

                # ---- QKV projection ----
                # first s-chunk: q/k kt-outer, consuming each xt piece as it
                # arrives from HBM instead of waiting for all 16.  Each head
                # packs its q (bank A) and k (bank B) groups in one psT tile.
                ssl = slice(0, 512)
                psqk = [psTp.tile([128, 1024], f32, tag="psT",
                                  name=f"psqk0_{b}_{h}") for h in range(HPC)]
                for kt in range(16):
                    for h in range(HPC):
                        nc.tensor.matmul(
                            psqk[h][:, 0:512], lhsT=wq_sb[h][:, kt, :],
                            rhs=xts[kt][:, ssl],
                            start=(kt == 0), stop=(kt == 15))
                        nc.tensor.matmul(
                            psqk[h][:, 512:1024], lhsT=wk_sb[h][:, kt, :],
                            rhs=xts[kt][:, ssl],
                            start=(kt == 0), stop=(kt == 15))
                for h in range(HPC):
                    nc.vector.tensor_scalar_add(q_sb[h][:, ssl],
                                                psqk[h][:, 0:512], bq_t[h][:])
                    nc.vector.tensor_scalar_add(k_sb[h][:, ssl],
                                                psqk[h][:, 512:1024], bk_t[h][:])
                # V st-outer (xt sc0 has fully landed by now); the four
                # 256-wide groups in one two-bank tile run sequentially —
                # two concurrently-open accumulation groups in one PSUM
                # bank corrupt the first term
                psv = psTp.tile([128, 1024], f32, tag="psT",
                                name=f"psv0_{b}")
                for st in range(4):
                    vsl = slice(st * 256, st * 256 + HPC * DH)
                    for kt in range(16):
                        nc.tensor.matmul(
                            psv[:, vsl],
                            lhsT=xts[kt][:, st * 128:(st + 1) * 128],
                            rhs=wv_sb[:, kt, :],
                            start=(kt == 0), stop=(kt == 15))
                    nc.vector.tensor_copy(v_sb[:, st, :, 0:DH], psv[:, vsl])
                # previous batch's trailing transpose/out-proj: the PSUM
                # banks are free again here, and these matmuls fill the
                # batch-boundary pipeline drain
                for fn in pending_tail:
                    fn()
                pending_tail = []
                for sc4 in range(1, 4):
                    ssl = slice(sc4 * 512, (sc4 + 1) * 512)
                    for h in range(HPC):
                        ps = psTp.tile([128, 1024], f32, tag="psT")
                        for kt in range(16):
                            nc.tensor.matmul(
                                ps[:, 0:512], lhsT=wq_sb[h][:, kt, :],
                                rhs=xts[kt][:, ssl],
                                start=(kt == 0), stop=(kt == 15))
                            nc.tensor.matmul(
                                ps[:, 512:1024], lhsT=wk_sb[h][:, kt, :],
                                rhs=xts[kt][:, ssl],
                                start=(kt == 0), stop=(kt == 15))
                        nc.vector.tensor_scalar_add(q_sb[h][:, ssl],
                                                    ps[:, 0:512], bq_t[h][:])
                        nc.vector.tensor_scalar_add(k_sb[h][:, ssl],
                                                    ps[:, 512:1024], bk_t[h][:])
                    for st4 in range(4):
                        st = sc4 * 4 + st4
                        tsl = slice(st * 128, (st + 1) * 128)
                        psv = psTp.tile([128, 1024], f32, tag="psT")
                        for kt in range(16):
                            nc.tensor.matmul(
                                psv[:, :HPC * DH], lhsT=xts[kt][:, tsl],
                                rhs=wv_sb[:, kt, :],
                                start=(kt == 0), stop=(kt == 15))
                        nc.vector.tensor_copy(v_sb[:, st, :, 0:DH],
                                              psv[:, :HPC * DH])

                # ---- attention ----
                def stage_scores(si, b=b, q_sb=q_sb, k_sb=k_sb):
                    """Transposed scores + exp for both heads of q-tile si.

                    Each two-bank PSUM chunk holds up to 4 j-blocks x 2
                    heads in [j, h, q] column order, so one ScalarE exp
                    covers the whole chunk.  Only the diagonal block gets
                    the causal mask; staircase blocks beyond the diagonal
                    are never consumed."""
                    nj = si + 1 if causal else NT
                    eT = expp.tile([128, nj, HPC, 128], bf16, tag="expT",
                                   name=f"expT_{b}_{si}")
                    for c4 in range(0, nj, 4):
                        jn = min(4, nj - c4)
                        psT = psTp.tile([128, 4, HPC, 128], f32, tag="psT",
                                        name=f"psT_{b}_{si}_{c4}")
                        for j4 in range(jn):
                            j = c4 + j4
                            for h in range(HPC):
                                nc.tensor.matmul(
                                    psT[:, j4, h, :],
                                    lhsT=k_sb[h][:, j * 128:(j + 1) * 128],
                                    rhs=q_sb[h][:, si * 128:(si + 1) * 128],
                                    start=True, stop=True)
                        if not causal:
                            mt = mbp.tile([128, 512], f32, tag="mb",
                                          name=f"mb_{b}_{si}_{c4}")
                            nc.sync.dma_start(
                                out=mt[:, :jn * 128],
                                in_=mb.ap()[c4 * 128:(c4 + jn) * 128,
                                            si * 128:(si + 1) * 128]
                                .rearrange("(j t) q -> t (j q)", t=128))
                            for h in range(HPC):
                                nc.vector.tensor_add(
                                    psT[:, :jn, h, :], psT[:, :jn, h, :],
                                    mt[:, :jn * 128])
                        nc.scalar.activation(
                            eT[:, c4:c4 + jn, :, :],
                            psT[:, :jn, :, :], AF.Exp, scale=SCALE)
                    if causal:
                        # zero the exp'd upper triangle of the diagonal block
                        # (identical to masking scores with -inf before exp)
                        for h in range(HPC):
                            nc.gpsimd.tensor_mul(eT[:, nj - 1, h, :],
                                                 eT[:, nj - 1, h, :], tri01[:])
                    return eT

                def stage_attnv(si, eT, b=b):
                    """attn@V with the ones-augmented V: PSUM [q, DH+1] where
                    column DH is the softmax denominator. Normalization =
                    VectorE reciprocal + per-partition tensor_scalar_mul."""
                    nj = si + 1 if causal else NT
                    po = pop.tile([128, HPC, DH + 4], f32, tag="po",
                                  name=f"po_{b}_{si}")  # [:, h, DH] = denom
                    o_h = []
                    for h in range(HPC):
                        for j in range(nj):
                            nc.tensor.matmul(
                                po[:, h, 0:DH + 1],
                                lhsT=eT[:, j, h, :],
                                rhs=v_sb[:, j, h, :],
                                start=(j == 0), stop=(j == nj - 1))
                        rinv = accp.tile([128, 1], f32, tag=f"rinv{h}",
                                         name=f"rinv_{b}_{si}_{h}")
                        nc.vector.reciprocal(rinv[:], po[:, h, DH:DH + 1])
                        o_sb = osbp.tile([128, DH], bf16, tag=f"osb{h}",
                                         name=f"osb_{b}_{si}_{h}")
                        nc.vector.tensor_scalar_mul(o_sb[:], po[:, h, 0:DH],
                                                    rinv[:])
                        o_h.append(o_sb)
                    return o_h

                def stage_transp(si, o_h, b=b):
                    """[q,dh]->[dh,q] PE transpose per head; bf16 eviction on
                    the gpsimd engine (scalar and vector are both loaded)."""
                    ptr = Ap.tile([128, 512], f32, tag="A",
                                  name=f"ptr_{b}_{si}")
                    for h in range(HPC):
                        nc.tensor.matmul(
                            ptr[:, h * DH:(h + 1) * DH],
                            lhsT=o_h[h][:], rhs=ident[:],
                            start=True, stop=True)
                    oT = outTp.tile([128, HPC * DH], bf16, tag="outT",
                                    name=f"oT_{b}_{si}")
                    nc.vector.tensor_copy(oT[:], ptr[:, :HPC * DH])
                    return oT

                def stage_oproj(si, oT, b=b):
                    """Out-projection with both heads in one PSUM group."""
                    ostage = outsp.tile([128, DM], f32, tag="ostage",
                                        name=f"ostage_{b}_{si}")
                    for ncn in range(4):
                        nsl = slice(ncn * 512, (ncn + 1) * 512)
                        pso = Ap.tile([128, 512], f32, tag="A",
                                      name=f"pso_{b}_{si}_{ncn}")
                        for h in range(HPC):
                            nc.tensor.matmul(pso[:],
                                             lhsT=oT[:, h * DH:(h + 1) * DH],
                                             rhs=wo_t[h][:, nsl],
                                             start=(h == 0), stop=(h == HPC - 1))
                        if ncn % 2 == 0:
                            nc.scalar.activation(ostage[:, nsl], pso[:],
                                                 AF.Copy)
                        else:
                            nc.vector.tensor_copy(ostage[:, nsl], pso[:])
                        last_tile = (si == NT - 1)
                        if last_tile:
                            # fine-grained stores so the tail drains early
                            nc.sync.dma_start(
                                out=outp.ap()[b, si * 128:(si + 1) * 128, nsl],
                                in_=ostage[:, nsl])
                        elif ncn % 2 == 1:
                            nc.sync.dma_start(
                                out=outp.ap()[b, si * 128:(si + 1) * 128,
                                              (ncn - 1) * 512:(ncn + 1) * 512],
                                in_=ostage[:, (ncn - 1) * 512:(ncn + 1) * 512])

                # software pipeline: scores two tiles ahead; transpose one
                # and out-proj two tiles behind attn@V
                exp_q = {0: stage_scores(0), 1: stage_scores(1)}
                o_box, oT_box = {}, {}
                for si in range(NT):
                    if si + 2 < NT:
                        exp_q[si + 2] = stage_scores(si + 2)
                    o_box[si] = stage_attnv(si, exp_q.pop(si))
                    if si >= 1:
                        oT_box[si - 1] = stage_transp(si - 1, o_box.pop(si - 1))
                    if si >= 2:
                        stage_oproj(si - 2, oT_box.pop(si - 2))
                pending_tail = [
                    (lambda si=NT - 1, oh=o_box.pop(NT - 1):
                     oT_box.__setitem__(si, stage_transp(si, oh))),
                    (lambda si=NT - 2: stage_oproj(si, oT_box.pop(si))),
                    (lambda si=NT - 1: stage_oproj(si, oT_box.pop(si))),
                ]

            # final batch's trailing stages
            for fn in pending_tail:
                fn()

    nc.compile()
    return nc


def _get(causal: bool):
    if causal not in _BUILT:
        _BUILT[causal] = _build(causal)
    return _BUILT[causal]


def _rot(fr, fi, m):
    """Apply the reference's per-head rotary as a linear map on rows of m."""
    top, bot = m[:DH // 2], m[DH // 2:]
    return np.concatenate([fr[:, None] * top - fi[:, None] * bot,
                           fi[:, None] * top + fr[:, None] * bot], axis=0)


def _pack(w):
    """[DM, C] -> [128, 16*C] with [p, kt*C+c] = w[kt*128+p, c]."""
    c = w.shape[1]
    return np.ascontiguousarray(
        w.reshape(16, 128, c).transpose(1, 0, 2).reshape(128, 16 * c))


def kernel(x, w_qkv, b_qkv, w_out, b_out, fc_real, fc_imag, mask):
    x = np.asarray(x, np.float32)
    w_qkv = np.asarray(w_qkv, np.float32)
    b_qkv = np.asarray(b_qkv, np.float32)
    w_out = np.asarray(w_out, np.float32)
    b_out = np.asarray(b_out, np.float32)
    fc_real = np.asarray(fc_real, np.float32)
    fc_imag = np.asarray(fc_imag, np.float32)
    mask_np = np.asarray(mask)[0, 0]

    causal = bool(np.array_equal(
        mask_np, np.triu(np.ones((S, S), bool), 1)))

    bf = ml_dtypes.bfloat16
    xt_host = np.ascontiguousarray(x.transpose(0, 2, 1)).astype(bf)

    in_maps = []
    maskb = None
    if not causal:
        # transposed ([t, q]) additive mask for the transposed-scores layout
        maskb = np.ascontiguousarray(
            np.where(mask_np, np.float32(-1e30), np.float32(0.0)).T)
    for c in range(NCORES):
        wq_h, wk_h, bq_h, bk_h, wv_h, wo_h = [], [], [], [], [], []
        for hh in range(HPC):
            g = c * HPC + hh
            fr = fc_real[0, g, :]
            fi = fc_imag[0, g, :]
            wq_h.append(_pack(
                _rot(fr, fi, w_qkv[g * DH:(g + 1) * DH, :]).T).astype(bf))
            wk_h.append(_pack(
                _rot(fr, fi, w_qkv[DM + g * DH:DM + (g + 1) * DH, :]).T
            ).astype(bf))
            bq_h.append(_rot(fr, fi, b_qkv[g * DH:(g + 1) * DH, None])[:, 0])
            bk_h.append(_rot(fr, fi,
                             b_qkv[DM + g * DH:DM + (g + 1) * DH, None])[:, 0])
            wv_h.append(w_qkv[2 * DM + g * DH:2 * DM + (g + 1) * DH, :].T)
            wo_h.append(np.ascontiguousarray(
                w_out[:, g * DH:(g + 1) * DH].T).astype(bf))
        m = {
            "xt": xt_host,
            "wq": np.stack(wq_h),
            "wk": np.stack(wk_h),
            "wv": _pack(np.concatenate(wv_h, axis=1)).astype(bf),
            "bq": np.stack(bq_h).astype(np.float32),
            "bk": np.stack(bk_h).astype(np.float32),
            "wo": np.stack(wo_h),
        }
        if not causal:
            m["maskb"] = maskb
        in_maps.append(m)

    from concourse.bass_utils import run_bass_kernel_spmd
    nc = _get(causal)
    global _LAST_IN_MAPS
    _LAST_IN_MAPS = in_maps
    trace = os.environ.get("MHA_TRACE") == "1"
    res = run_bass_kernel_spmd(nc, in_maps, core_ids=list(range(NCORES)),
                               trace=trace)
    if trace:
        kernel.last_results = res

    out = res.results[0]["outp"].astype(np.float64)
    for c in range(1, NCORES):
        out += res.results[c]["outp"]
    b_v = b_qkv[2 * DM:]
    out += (b_out + w_out @ b_v)[None, None, :]
    return out.astype(np.float32)


# revision 23
# speedup vs baseline: 1.1626x; 1.0184x over previous
"""Trainium2 Bass kernel for nn_MultiHeadAttention_9405978378694.

Full-input contract: kernel(**inputs) -> (B, S, DM) float32.

Sharding: tensor-parallel over heads. 16 heads / 8 cores = 2 heads per core.
Each core computes QKV projection for its heads (full sequence, both
batches), causal attention, and a partial out-projection against its slice
of w_out columns. Partials are summed on the host (the all-reduce).

Host-side algebra (exact):
  - The reference applies rotary with frequencies indexed by the HEAD axis
    (not position), so each head's rotation is a constant 128x128 linear map.
    It is folded into the Q/K projection weights: Wq' = R_h @ Wq_h.
  - The V bias enters the output as attn_rows_sum(=1) * b_v, which passes
    linearly through the out-projection: folded into the final bias add as
    w_out @ b_v.
  - Weights are packed host-side so every SBUF partition's data is
    contiguous in HBM (4KB runs instead of 256B scatters).

Device kernel (per core, all matmuls bf16 with fp32 PSUM accumulation):
  QKV in transposed [dh, s] layout -> scores computed DIRECTLY TRANSPOSED
  (lhsT = k-block, rhs = q-tile -> PSUM [t, q]), so no PE transpose of the
  attention matrix is ever needed -> both heads' score blocks packed in one
  two-bank PSUM chunk so a single ScalarE exp covers up to 1024 columns ->
  attn@V with lhsT = expT block and rhs = V augmented with a ones column,
  giving out [q, dh] AND the softmax denominator in column dh of the same
  PSUM tile -> VectorE reciprocal + per-partition tensor_scalar_mul does
  the normalization -> one [q,dh]->[dh,q] PE transpose per (tile, head) ->
  out-projection with both heads in one PSUM group.

Engine balance: ScalarE runs ONLY the exps (it is the softmax throughput
limit at ~1 col/cycle); all PSUM evictions (q/k with bias, V, out-proj
stage) run on VectorE; oT evictions and all output DMAs ride the
otherwise-idle GpSimd queue.

Scheduling notes:
  - per query tile si the PE emission order is scores(si+2), attn@V(si),
    transpose(si-1), out-proj(si-2): the exp chain and the
    reciprocal/normalize/oT-evict chains each get a full iteration of PE
    work to drain, so nothing stalls the PE in steady state;
  - batch 0's trailing transpose/out-proj stages are deferred into batch
    1's QKV section so the pipeline-drain bubble at the batch boundary is
    filled with projection matmuls;
  - startup: whole-tile weight DMAs (one per head per projection) then
    per-kt xt chunks on two alternating HWDGE queues; the kt-outer QKV
    accumulation starts as soon as the four weight tiles + first xt chunk
    land (~1.2MB);
  - batch 1's xt is prefetched on the gpsimd queue during batch 0's
    attention, gated per-tile by WAR on batch 0's QKV reads.
"""

import os
import numpy as np
import ml_dtypes

B, S, DM, H, DH = 2, 2048, 2048, 16, 128
NCORES = 8
HPC = H // NCORES  # heads per core
NT = S // 128      # 128-row tiles along sequence
SCALE = float(DH) ** -0.5

_BUILT = {}
_LAST_IN_MAPS = None


def _build(causal: bool):
    import concourse.mybir as mybir
    import concourse.tile as tile
    from concourse import bacc
    from concourse.masks import make_identity

    f32 = mybir.dt.float32
    bf16 = mybir.dt.bfloat16
    AF = mybir.ActivationFunctionType

    nc = bacc.Bacc("TRN2", target_bir_lowering=False, debug=False)

    xt = nc.dram_tensor("xt", [B, DM, S], bf16, kind="ExternalInput")
    # partition-contiguous packed weights: [p, kt, m] = w[kt*128+p, m]
    wq = nc.dram_tensor("wq", [HPC, 128, 16 * DH], bf16, kind="ExternalInput")
    wk = nc.dram_tensor("wk", [HPC, 128, 16 * DH], bf16, kind="ExternalInput")
    wv = nc.dram_tensor("wv", [128, 16 * HPC * DH], bf16, kind="ExternalInput")
    bq = nc.dram_tensor("bq", [HPC, DH], f32, kind="ExternalInput")
    bk = nc.dram_tensor("bk", [HPC, DH], f32, kind="ExternalInput")
    wo = nc.dram_tensor("wo", [HPC, DH, DM], bf16, kind="ExternalInput")
    if not causal:
        # host sends the mask TRANSPOSED ([t, q] orientation)
        mb = nc.dram_tensor("maskb", [S, S], f32, kind="ExternalInput")
    # bf16 partials: the host sums 8 of them in float64; the 0.4% rounding
    # adds ~sqrt-sum to a 5.3e-3 baseline error, far under the 2e-2 gate,
    # and halves the 33.5MB/core output stream
    outp = nc.dram_tensor("outp", [B, S, DM], bf16, kind="ExternalOutput")

    from contextlib import ExitStack
    with tile.TileContext(nc) as tc:
        with ExitStack() as es:
            constp = es.enter_context(tc.tile_pool(name="const", bufs=1))
            wqkp = es.enter_context(tc.tile_pool(name="wqk", bufs=1))
            wvp = es.enter_context(tc.tile_pool(name="wvp", bufs=1))
            wop = es.enter_context(tc.tile_pool(name="wop", bufs=1))
            xtp = es.enter_context(tc.tile_pool(name="xtp", bufs=1))
            qkvp = es.enter_context(tc.tile_pool(name="qkv", bufs=1))
            expp = es.enter_context(tc.tile_pool(name="expT", bufs=3))
            accp = es.enter_context(tc.tile_pool(name="accs", bufs=4))
            osbp = es.enter_context(tc.tile_pool(name="osb", bufs=2))
            outTp = es.enter_context(tc.tile_pool(name="outT", bufs=2))
            outsp = es.enter_context(tc.tile_pool(name="ostage", bufs=2))
            mbp = es.enter_context(tc.tile_pool(name="mbp", bufs=3))
            # PSUM budget (8 banks of 2KB/partition):
            #   A   2 x [128,512]f32   = 2 banks (out-proj, transposes, warmup)
            #   psT 2 x [128,1024]f32  = 4 banks (scores chunks, q|k packed
            #                                     per head, V-proj groups)
            #   po  2 x [128,2,132]f32 = 2 banks (attn@V out + softmax denom)
            Ap = es.enter_context(tc.tile_pool(name="A", bufs=2, space="PSUM"))
            psTp = es.enter_context(tc.tile_pool(name="psT", bufs=2, space="PSUM"))
            pop = es.enter_context(tc.tile_pool(name="po", bufs=2, space="PSUM"))

            ident = constp.tile([128, 128], bf16)
            make_identity(nc, ident[:])
            warm_src = constp.tile([128, 512], bf16)
            nc.gpsimd.memset(warm_src[:], 0.5)
            # causal 0/1 triangle for the transposed-diagonal block: the
            # diagonal is masked AFTER exp by zeroing eT on the gpsimd
            # engine (SBUF-only; gpsimd cannot touch PSUM), which keeps
            # the score->exp chain free of extra PSUM ops
            tri01 = constp.tile([128, 128], bf16)
            nc.gpsimd.memset(tri01[:], 1.0)
            nc.gpsimd.affine_select(
                out=tri01[:], in_=tri01[:],
                compare_op=mybir.AluOpType.is_ge, fill=0.0,
                base=0, pattern=[[1, 128]], channel_multiplier=-1,
            )

            wq_sb = [wqkp.tile([128, 16, DH], bf16, tag=f"wq{h}", name=f"wq{h}")
                     for h in range(HPC)]
            wk_sb = [wqkp.tile([128, 16, DH], bf16, tag=f"wk{h}", name=f"wk{h}")
                     for h in range(HPC)]
            wv_sb = wvp.tile([128, 16, HPC * DH], bf16, tag="wv", name="wv")
            wo_t = [wop.tile([128, DM], bf16, tag=f"wo{h}", name=f"wo{h}") for h in range(HPC)]
            bq_t = [constp.tile([128, 1], f32, tag=f"bq{h}", name=f"bq{h}") for h in range(HPC)]
            bk_t = [constp.tile([128, 1], f32, tag=f"bk{h}", name=f"bk{h}") for h in range(HPC)]

            # V for both heads with a ones column per (j, h):
            # [t_local, j, h, 0:128]=V, [t_local, j, h, 128]=1.0 (softmax denom)
            v_sb = qkvp.tile([128, NT, HPC, DH + 1], bf16, tag="v", name="v_aug")
            nc.gpsimd.memset(v_sb[:, :, :, DH:DH + 1], 1.0)

            # dummy matmuls bridge the PE to the first weight/xt arrivals so
            # the p-state is ramped when real work starts
            warm_ps = Ap.tile([128, 512], f32, tag="A", name="warm_ps")
            for _ in range(28):
                nc.tensor.matmul(warm_ps[:], lhsT=ident[:], rhs=warm_src[:],
                                 start=True, stop=True)
            for h in range(HPC):
                nc.gpsimd.dma_start(out=bq_t[h][:],
                                    in_=bq.ap()[h].rearrange("(d o) -> d o", o=1))
                nc.gpsimd.dma_start(out=bk_t[h][:],
                                    in_=bk.ap()[h].rearrange("(d o) -> d o", o=1))

            xts0 = [xtp.tile([128, S], bf16, tag=f"xt{kt}", name=f"xt_0_{kt}")
                    for kt in range(16)]
            # startup across all three HWDGE queues (~140GB/s each):
            # sync/scalar carry the weights + the sc0/sc1 xt chunks the
            # kt-outer and first sc-outer loops need; gpsimd carries the
            # sc2+sc3 xt halves needed last
            nc.sync.dma_start(out=wq_sb[0][:], in_=wq.ap()[0])
            nc.scalar.dma_start(out=wk_sb[0][:], in_=wk.ap()[0])
            nc.sync.dma_start(out=wq_sb[1][:], in_=wq.ap()[1])
            nc.scalar.dma_start(out=wk_sb[1][:], in_=wk.ap()[1])
            nc.scalar.dma_start(out=wv_sb[:], in_=wv.ap()[:])
            for kt in range(16):
                eng = nc.sync if kt % 2 == 0 else nc.scalar
                eng.dma_start(
                    out=xts0[kt][:, 0:512],
                    in_=xt.ap()[0, kt * 128:(kt + 1) * 128, 0:512])
            for kt in range(16):
                eng = nc.sync if kt % 2 == 0 else nc.scalar
                eng.dma_start(
                    out=xts0[kt][:, 512:1024],
                    in_=xt.ap()[0, kt * 128:(kt + 1) * 128, 512:1024])
            for kt in range(16):
                nc.gpsimd.dma_start(
                    out=xts0[kt][:, 1024:2048],
                    in_=xt.ap()[0, kt * 128:(kt + 1) * 128, 1024:2048])
            nc.sync.dma_start(out=wo_t[0][:], in_=wo.ap()[0])
            nc.scalar.dma_start(out=wo_t[1][:], in_=wo.ap()[1])

            pending_tail = []  # deferred trailing stages of the previous batch

            for b in range(B):
                if b == 0:
                    xts = xts0
                else:
                    # prefetched on the gpsimd queue during batch 0's
                    # attention; WAR deps gate each tile on batch 0's QKV
                    xts = [xtp.tile([128, S], bf16, tag=f"xt{kt}",
                                    name=f"xt_1_{kt}")
                           for kt in range(16)]
                    for kt in range(16):
                        nc.gpsimd.dma_start(
                            out=xts[kt][:],
                            in_=xt.ap()[b, kt * 128:(kt + 1) * 128, :])

                q_sb = [qkvp.tile([128, S], bf16, tag=f"q{h}", name=f"q_{b}_{h}") for h in range(HPC)]
                k_sb = [qkvp.tile([128, S], bf16, tag=f"k{h}", name=f"k_{b}_{h}") for h in range(HPC)]

                # ---- QKV projection ----
                # first s-chunk: q/k kt-outer, consuming each xt piece as it
                # arrives from HBM instead of waiting for all 16.  Each head
                # packs its q (bank A) and k (bank B) groups in one psT tile.
                ssl = slice(0, 512)
                psqk = [psTp.tile([128, 1024], f32, tag="psT",
                                  name=f"psqk0_{b}_{h}") for h in range(HPC)]
                for kt in range(16):
                    for h in range(HPC):
                        nc.tensor.matmul(
                            psqk[h][:, 0:512], lhsT=wq_sb[h][:, kt, :],
                            rhs=xts[kt][:, ssl],
                            start=(kt == 0), stop=(kt == 15))
                        nc.tensor.matmul(
                            psqk[h][:, 512:1024], lhsT=wk_sb[h][:, kt, :],
                            rhs=xts[kt][:, ssl],
                            start=(kt == 0), stop=(kt == 15))
                for h in range(HPC):
                    nc.vector.tensor_scalar_add(q_sb[h][:, ssl],
                                                psqk[h][:, 0:512], bq_t[h][:])
                    nc.vector.tensor_scalar_add(k_sb[h][:, ssl],
                                                psqk[h][:, 512:1024], bk_t[h][:])
                # V st-outer (xt sc0 has fully landed by now); the four
                # 256-wide groups in one two-bank tile run sequentially —
                # two concurrently-open accumulation groups in one PSUM
                # bank corrupt the first term
                psv = psTp.tile([128, 1024], f32, tag="psT",
                                name=f"psv0_{b}")
                for st in range(4):
                    vsl = slice(st * 256, st * 256 + HPC * DH)
                    for kt in range(16):
                        nc.tensor.matmul(
                            psv[:, vsl],
                            lhsT=xts[kt][:, st * 128:(st + 1) * 128],
                            rhs=wv_sb[:, kt, :],
                            start=(kt == 0), stop=(kt == 15))
                    nc.vector.tensor_copy(v_sb[:, st, :, 0:DH], psv[:, vsl])
                # previous batch's trailing transpose/out-proj: the PSUM
                # banks are free again here, and these matmuls fill the
                # batch-boundary pipeline drain
                for fn in pending_tail:
                    fn()
                pending_tail = []
                for sc4 in range(1, 4):
                    ssl = slice(sc4 * 512, (sc4 + 1) * 512)
                    for h in range(HPC):
                        ps = psTp.tile([128, 1024], f32, tag="psT")
                        for kt in range(16):
                            nc.tensor.matmul(
                                ps[:, 0:512], lhsT=wq_sb[h][:, kt, :],
                                rhs=xts[kt][:, ssl],
                                start=(kt == 0), stop=(kt == 15))
                            nc.tensor.matmul(
                                ps[:, 512:1024], lhsT=wk_sb[h][:, kt, :],
                                rhs=xts[kt][:, ssl],
                                start=(kt == 0), stop=(kt == 15))
                        nc.vector.tensor_scalar_add(q_sb[h][:, ssl],
                                                    ps[:, 0:512], bq_t[h][:])
                        nc.vector.tensor_scalar_add(k_sb[h][:, ssl],
                                                    ps[:, 512:1024], bk_t[h][:])
                    for st4 in range(4):
                        st = sc4 * 4 + st4
                        tsl = slice(st * 128, (st + 1) * 128)
                        psv = psTp.tile([128, 1024], f32, tag="psT")
                        for kt in range(16):
                            nc.tensor.matmul(
                                psv[:, :HPC * DH], lhsT=xts[kt][:, tsl],
                                rhs=wv_sb[:, kt, :],
                                start=(kt == 0), stop=(kt == 15))
                        nc.vector.tensor_copy(v_sb[:, st, :, 0:DH],
                                              psv[:, :HPC * DH])

                # ---- attention ----
                def stage_scores(si, b=b, q_sb=q_sb, k_sb=k_sb):
                    """Transposed scores + exp for both heads of q-tile si.

                    Each two-bank PSUM chunk holds up to 4 j-blocks x 2
                    heads in [j, h, q] column order, so one ScalarE exp
                    covers the whole chunk.  Only the diagonal block gets
                    the causal mask; staircase blocks beyond the diagonal
                    are never consumed."""
                    nj = si + 1 if causal else NT
                    eT = expp.tile([128, nj, HPC, 128], bf16, tag="expT",
                                   name=f"expT_{b}_{si}")
                    for c4 in range(0, nj, 4):
                        jn = min(4, nj - c4)
                        psT = psTp.tile([128, 4, HPC, 128], f32, tag="psT",
                                        name=f"psT_{b}_{si}_{c4}")
                        for j4 in range(jn):
                            j = c4 + j4
                            for h in range(HPC):
                                nc.tensor.matmul(
                                    psT[:, j4, h, :],
                                    lhsT=k_sb[h][:, j * 128:(j + 1) * 128],
                                    rhs=q_sb[h][:, si * 128:(si + 1) * 128],
                                    start=True, stop=True)
                        if not causal:
                            mt = mbp.tile([128, 512], f32, tag="mb",
                                          name=f"mb_{b}_{si}_{c4}")
                            nc.sync.dma_start(
                                out=mt[:, :jn * 128],
                                in_=mb.ap()[c4 * 128:(c4 + jn) * 128,
                                            si * 128:(si + 1) * 128]
                                .rearrange("(j t) q -> t (j q)", t=128))
                            for h in range(HPC):
                                nc.vector.tensor_add(
                                    psT[:, :jn, h, :], psT[:, :jn, h, :],
                                    mt[:, :jn * 128])
                        nc.scalar.activation(
                            eT[:, c4:c4 + jn, :, :],
                            psT[:, :jn, :, :], AF.Exp, scale=SCALE)
                    if causal:
                        # zero the exp'd upper triangle of the diagonal block
                        # (identical to masking scores with -inf before exp)
                        for h in range(HPC):
                            nc.gpsimd.tensor_mul(eT[:, nj - 1, h, :],
                                                 eT[:, nj - 1, h, :], tri01[:])
                    return eT

                def stage_attnv(si, eT, b=b):
                    """attn@V with the ones-augmented V: PSUM [q, DH+1] where
                    column DH is the softmax denominator. Normalization =
                    VectorE reciprocal + per-partition tensor_scalar_mul."""
                    nj = si + 1 if causal else NT
                    po = pop.tile([128, HPC, DH + 4], f32, tag="po",
                                  name=f"po_{b}_{si}")  # [:, h, DH] = denom
                    o_h = []
                    for h in range(HPC):
                        for j in range(nj):
                            nc.tensor.matmul(
                                po[:, h, 0:DH + 1],
                                lhsT=eT[:, j, h, :],
                                rhs=v_sb[:, j, h, :],
                                start=(j == 0), stop=(j == nj - 1))
                        rinv = accp.tile([128, 1], f32, tag=f"rinv{h}",
                                         name=f"rinv_{b}_{si}_{h}")
                        nc.vector.reciprocal(rinv[:], po[:, h, DH:DH + 1])
                        o_sb = osbp.tile([128, DH], bf16, tag=f"osb{h}",
                                         name=f"osb_{b}_{si}_{h}")
                        nc.vector.tensor_scalar_mul(o_sb[:], po[:, h, 0:DH],
                                                    rinv[:])
                        o_h.append(o_sb)
                    return o_h

                def stage_transp(si, o_h, b=b):
                    """[q,dh]->[dh,q] PE transpose per head; bf16 eviction on
                    the gpsimd engine (scalar and vector are both loaded)."""
                    ptr = Ap.tile([128, 512], f32, tag="A",
                                  name=f"ptr_{b}_{si}")
                    for h in range(HPC):
                        nc.tensor.matmul(
                            ptr[:, h * DH:(h + 1) * DH],
                            lhsT=o_h[h][:], rhs=ident[:],
                            start=True, stop=True)
                    oT = outTp.tile([128, HPC * DH], bf16, tag="outT",
                                    name=f"oT_{b}_{si}")
                    nc.vector.tensor_copy(oT[:], ptr[:, :HPC * DH])
                    return oT

                def stage_oproj(si, oT, b=b):
                    """Out-projection with both heads in one PSUM group.
                    Output stores alternate between the sync and gpsimd
                    queues so neither carries the whole output stream."""
                    oeng = nc.sync if si % 2 == 0 else nc.gpsimd
                    ostage = outsp.tile([128, DM], bf16, tag="ostage",
                                        name=f"ostage_{b}_{si}")
                    for ncn in range(4):
                        nsl = slice(ncn * 512, (ncn + 1) * 512)
                        pso = Ap.tile([128, 512], f32, tag="A",
                                      name=f"pso_{b}_{si}_{ncn}")
                        for h in range(HPC):
                            nc.tensor.matmul(pso[:],
                                             lhsT=oT[:, h * DH:(h + 1) * DH],
                                             rhs=wo_t[h][:, nsl],
                                             start=(h == 0), stop=(h == HPC - 1))
                        if ncn % 2 == 0:
                            nc.scalar.activation(ostage[:, nsl], pso[:],
                                                 AF.Copy)
                        else:
                            nc.vector.tensor_copy(ostage[:, nsl], pso[:])
                        last_tile = (si == NT - 1)
                        if last_tile:
                            # fine-grained stores so the tail drains early
                            oeng.dma_start(
                                out=outp.ap()[b, si * 128:(si + 1) * 128, nsl],
                                in_=ostage[:, nsl])
                        elif ncn % 2 == 1:
                            oeng.dma_start(
                                out=outp.ap()[b, si * 128:(si + 1) * 128,
                                              (ncn - 1) * 512:(ncn + 1) * 512],
                                in_=ostage[:, (ncn - 1) * 512:(ncn + 1) * 512])

                # software pipeline: scores two tiles ahead; transpose one
                # and out-proj two tiles behind attn@V
                exp_q = {0: stage_scores(0), 1: stage_scores(1)}
                o_box, oT_box = {}, {}
                for si in range(NT):
                    if si + 2 < NT:
                        exp_q[si + 2] = stage_scores(si + 2)
                    o_box[si] = stage_attnv(si, exp_q.pop(si))
                    if si >= 1:
                        oT_box[si - 1] = stage_transp(si - 1, o_box.pop(si - 1))
                    if si >= 2:
                        stage_oproj(si - 2, oT_box.pop(si - 2))
                pending_tail = [
                    (lambda si=NT - 1, oh=o_box.pop(NT - 1):
                     oT_box.__setitem__(si, stage_transp(si, oh))),
                    (lambda si=NT - 2: stage_oproj(si, oT_box.pop(si))),
                    (lambda si=NT - 1: stage_oproj(si, oT_box.pop(si))),
                ]

            # final batch's trailing stages
            for fn in pending_tail:
                fn()

    nc.compile()
    return nc


def _get(causal: bool):
    if causal not in _BUILT:
        _BUILT[causal] = _build(causal)
    return _BUILT[causal]


def _rot(fr, fi, m):
    """Apply the reference's per-head rotary as a linear map on rows of m."""
    top, bot = m[:DH // 2], m[DH // 2:]
    return np.concatenate([fr[:, None] * top - fi[:, None] * bot,
                           fi[:, None] * top + fr[:, None] * bot], axis=0)


def _pack(w):
    """[DM, C] -> [128, 16*C] with [p, kt*C+c] = w[kt*128+p, c]."""
    c = w.shape[1]
    return np.ascontiguousarray(
        w.reshape(16, 128, c).transpose(1, 0, 2).reshape(128, 16 * c))


def kernel(x, w_qkv, b_qkv, w_out, b_out, fc_real, fc_imag, mask):
    x = np.asarray(x, np.float32)
    w_qkv = np.asarray(w_qkv, np.float32)
    b_qkv = np.asarray(b_qkv, np.float32)
    w_out = np.asarray(w_out, np.float32)
    b_out = np.asarray(b_out, np.float32)
    fc_real = np.asarray(fc_real, np.float32)
    fc_imag = np.asarray(fc_imag, np.float32)
    mask_np = np.asarray(mask)[0, 0]

    causal = bool(np.array_equal(
        mask_np, np.triu(np.ones((S, S), bool), 1)))

    bf = ml_dtypes.bfloat16
    xt_host = np.ascontiguousarray(x.transpose(0, 2, 1)).astype(bf)

    in_maps = []
    maskb = None
    if not causal:
        # transposed ([t, q]) additive mask for the transposed-scores layout
        maskb = np.ascontiguousarray(
            np.where(mask_np, np.float32(-1e30), np.float32(0.0)).T)
    for c in range(NCORES):
        wq_h, wk_h, bq_h, bk_h, wv_h, wo_h = [], [], [], [], [], []
        for hh in range(HPC):
            g = c * HPC + hh
            fr = fc_real[0, g, :]
            fi = fc_imag[0, g, :]
            wq_h.append(_pack(
                _rot(fr, fi, w_qkv[g * DH:(g + 1) * DH, :]).T).astype(bf))
            wk_h.append(_pack(
                _rot(fr, fi, w_qkv[DM + g * DH:DM + (g + 1) * DH, :]).T
            ).astype(bf))
            bq_h.append(_rot(fr, fi, b_qkv[g * DH:(g + 1) * DH, None])[:, 0])
            bk_h.append(_rot(fr, fi,
                             b_qkv[DM + g * DH:DM + (g + 1) * DH, None])[:, 0])
            wv_h.append(w_qkv[2 * DM + g * DH:2 * DM + (g + 1) * DH, :].T)
            wo_h.append(np.ascontiguousarray(
                w_out[:, g * DH:(g + 1) * DH].T).astype(bf))
        m = {
            "xt": xt_host,
            "wq": np.stack(wq_h),
            "wk": np.stack(wk_h),
            "wv": _pack(np.concatenate(wv_h, axis=1)).astype(bf),
            "bq": np.stack(bq_h).astype(np.float32),
            "bk": np.stack(bk_h).astype(np.float32),
            "wo": np.stack(wo_h),
        }
        if not causal:
            m["maskb"] = maskb
        in_maps.append(m)

    from concourse.bass_utils import run_bass_kernel_spmd
    nc = _get(causal)
    global _LAST_IN_MAPS
    _LAST_IN_MAPS = in_maps
    trace = os.environ.get("MHA_TRACE") == "1"
    res = run_bass_kernel_spmd(nc, in_maps, core_ids=list(range(NCORES)),
                               trace=trace)
    if trace:
        kernel.last_results = res

    out = res.results[0]["outp"].astype(np.float64)
    for c in range(1, NCORES):
        out += res.results[c]["outp"]
    b_v = b_qkv[2 * DM:]
    out += (b_out + w_out @ b_v)[None, None, :]
    return out.astype(np.float32)


# revision 25
# speedup vs baseline: 1.2002x; 1.0324x over previous
"""Trainium2 Bass kernel for nn_MultiHeadAttention_9405978378694.

Full-input contract: kernel(**inputs) -> (B, S, DM) float32.

Sharding: tensor-parallel over heads. 16 heads / 8 cores = 2 heads per core.
Each core computes QKV projection for its heads (full sequence, both
batches), causal attention, and a partial out-projection against its slice
of w_out columns. Partials are summed on the host (the all-reduce).

Host-side algebra (exact):
  - The reference applies rotary with frequencies indexed by the HEAD axis
    (not position), so each head's rotation is a constant 128x128 linear map.
    It is folded into the Q/K projection weights: Wq' = R_h @ Wq_h.
  - The V bias enters the output as attn_rows_sum(=1) * b_v, which passes
    linearly through the out-projection: folded into the final bias add as
    w_out @ b_v.
  - Weights are packed host-side so every SBUF partition's data is
    contiguous in HBM (4KB runs instead of 256B scatters).

Device kernel (per core, all matmuls bf16 with fp32 PSUM accumulation):
  QKV in transposed [dh, s] layout -> scores computed DIRECTLY TRANSPOSED
  (lhsT = k-block, rhs = q-tile -> PSUM [t, q]), so no PE transpose of the
  attention matrix is ever needed -> both heads' score blocks packed in one
  two-bank PSUM chunk so a single ScalarE exp covers up to 1024 columns ->
  attn@V with lhsT = expT block and rhs = V augmented with a ones column,
  giving out [q, dh] AND the softmax denominator in column dh of the same
  PSUM tile -> VectorE reciprocal + per-partition tensor_scalar_mul does
  the normalization -> one [q,dh]->[dh,q] PE transpose per (tile, head) ->
  out-projection with both heads in one PSUM group.

Engine balance: ScalarE runs ONLY the exps (it is the softmax throughput
limit at ~1 col/cycle); all PSUM evictions (q/k with bias, V, out-proj
stage) run on VectorE; oT evictions and all output DMAs ride the
otherwise-idle GpSimd queue.

Scheduling notes:
  - per query tile si the PE emission order is scores(si+2), attn@V(si),
    transpose(si-1), out-proj(si-2): the exp chain and the
    reciprocal/normalize/oT-evict chains each get a full iteration of PE
    work to drain, so nothing stalls the PE in steady state;
  - batch 0's trailing transpose/out-proj stages are deferred into batch
    1's QKV section so the pipeline-drain bubble at the batch boundary is
    filled with projection matmuls;
  - startup: whole-tile weight DMAs (one per head per projection) then
    per-kt xt chunks on two alternating HWDGE queues; the kt-outer QKV
    accumulation starts as soon as the four weight tiles + first xt chunk
    land (~1.2MB);
  - batch 1's xt is prefetched on the gpsimd queue during batch 0's
    attention, gated per-tile by WAR on batch 0's QKV reads.
"""

import os
import numpy as np
import ml_dtypes

B, S, DM, H, DH = 2, 2048, 2048, 16, 128
NCORES = 8
HPC = H // NCORES  # heads per core
NT = S // 128      # 128-row tiles along sequence
SCALE = float(DH) ** -0.5

_BUILT = {}
_LAST_IN_MAPS = None


def _build(causal: bool):
    import concourse.mybir as mybir
    import concourse.tile as tile
    from concourse import bacc
    from concourse.masks import make_identity

    f32 = mybir.dt.float32
    bf16 = mybir.dt.bfloat16
    AF = mybir.ActivationFunctionType

    nc = bacc.Bacc("TRN2", target_bir_lowering=False, debug=False)

    xt = nc.dram_tensor("xt", [B, DM, S], bf16, kind="ExternalInput")
    # partition-contiguous packed weights: [p, kt, m] = w[kt*128+p, m]
    wq = nc.dram_tensor("wq", [HPC, 128, 16 * DH], bf16, kind="ExternalInput")
    wk = nc.dram_tensor("wk", [HPC, 128, 16 * DH], bf16, kind="ExternalInput")
    wv = nc.dram_tensor("wv", [128, 16 * HPC * DH], bf16, kind="ExternalInput")
    bq = nc.dram_tensor("bq", [HPC, DH], f32, kind="ExternalInput")
    bk = nc.dram_tensor("bk", [HPC, DH], f32, kind="ExternalInput")
    wo = nc.dram_tensor("wo", [HPC, DH, DM], bf16, kind="ExternalInput")
    if not causal:
        # host sends the mask TRANSPOSED ([t, q] orientation)
        mb = nc.dram_tensor("maskb", [S, S], f32, kind="ExternalInput")
    # bf16 partials: the host sums 8 of them in float64; the 0.4% rounding
    # adds ~sqrt-sum to a 5.3e-3 baseline error, far under the 2e-2 gate,
    # and halves the 33.5MB/core output stream
    outp = nc.dram_tensor("outp", [B, S, DM], bf16, kind="ExternalOutput")

    from contextlib import ExitStack
    with tile.TileContext(nc) as tc:
        with ExitStack() as es:
            constp = es.enter_context(tc.tile_pool(name="const", bufs=1))
            wqkp = es.enter_context(tc.tile_pool(name="wqk", bufs=1))
            wvp = es.enter_context(tc.tile_pool(name="wvp", bufs=1))
            wop = es.enter_context(tc.tile_pool(name="wop", bufs=1))
            xtp = es.enter_context(tc.tile_pool(name="xtp", bufs=1))
            qkvp = es.enter_context(tc.tile_pool(name="qkv", bufs=1))
            expp = es.enter_context(tc.tile_pool(name="expT", bufs=3))
            accp = es.enter_context(tc.tile_pool(name="accs", bufs=4))
            osbp = es.enter_context(tc.tile_pool(name="osb", bufs=2))
            outTp = es.enter_context(tc.tile_pool(name="outT", bufs=2))
            outsp = es.enter_context(tc.tile_pool(name="ostage", bufs=2))
            mbp = es.enter_context(tc.tile_pool(name="mbp", bufs=3))
            # PSUM budget (8 banks of 2KB/partition):
            #   A   2 x [128,512]f32   = 2 banks (out-proj, transposes, warmup)
            #   psT 2 x [128,1024]f32  = 4 banks (scores chunks, q|k packed
            #                                     per head, V-proj groups)
            #   po  2 x [128,2,132]f32 = 2 banks (attn@V out + softmax denom)
            Ap = es.enter_context(tc.tile_pool(name="A", bufs=2, space="PSUM"))
            psTp = es.enter_context(tc.tile_pool(name="psT", bufs=2, space="PSUM"))
            pop = es.enter_context(tc.tile_pool(name="po", bufs=2, space="PSUM"))

            ident = constp.tile([128, 128], bf16)
            make_identity(nc, ident[:])
            warm_src = constp.tile([128, 512], bf16)
            nc.gpsimd.memset(warm_src[:], 0.5)
            # causal 0/1 triangle for the transposed-diagonal block: the
            # diagonal is masked AFTER exp by zeroing eT on the gpsimd
            # engine (SBUF-only; gpsimd cannot touch PSUM), which keeps
            # the score->exp chain free of extra PSUM ops
            tri01 = constp.tile([128, 128], bf16)
            nc.gpsimd.memset(tri01[:], 1.0)
            nc.gpsimd.affine_select(
                out=tri01[:], in_=tri01[:],
                compare_op=mybir.AluOpType.is_ge, fill=0.0,
                base=0, pattern=[[1, 128]], channel_multiplier=-1,
            )

            wq_sb = [wqkp.tile([128, 16, DH], bf16, tag=f"wq{h}", name=f"wq{h}")
                     for h in range(HPC)]
            wk_sb = [wqkp.tile([128, 16, DH], bf16, tag=f"wk{h}", name=f"wk{h}")
                     for h in range(HPC)]
            wv_sb = wvp.tile([128, 16, HPC * DH], bf16, tag="wv", name="wv")
            wo_t = [wop.tile([128, DM], bf16, tag=f"wo{h}", name=f"wo{h}") for h in range(HPC)]
            bq_t = [constp.tile([128, 1], f32, tag=f"bq{h}", name=f"bq{h}") for h in range(HPC)]
            bk_t = [constp.tile([128, 1], f32, tag=f"bk{h}", name=f"bk{h}") for h in range(HPC)]

            # V for both heads with a ones column per (j, h):
            # [t_local, j, h, 0:128]=V, [t_local, j, h, 128]=1.0 (softmax denom)
            v_sb = qkvp.tile([128, NT, HPC, DH + 1], bf16, tag="v", name="v_aug")
            nc.gpsimd.memset(v_sb[:, :, :, DH:DH + 1], 1.0)

            # dummy matmuls bridge the PE to the first weight/xt arrivals so
            # the p-state is ramped when real work starts
            warm_ps = Ap.tile([128, 512], f32, tag="A", name="warm_ps")
            for _ in range(28):
                nc.tensor.matmul(warm_ps[:], lhsT=ident[:], rhs=warm_src[:],
                                 start=True, stop=True)
            for h in range(HPC):
                nc.gpsimd.dma_start(out=bq_t[h][:],
                                    in_=bq.ap()[h].rearrange("(d o) -> d o", o=1))
                nc.gpsimd.dma_start(out=bk_t[h][:],
                                    in_=bk.ap()[h].rearrange("(d o) -> d o", o=1))

            xts0 = [xtp.tile([128, S], bf16, tag=f"xt{kt}", name=f"xt_0_{kt}")
                    for kt in range(16)]
            # startup across all three HWDGE queues (~140GB/s each):
            # sync/scalar carry the weights + the sc0/sc1 xt chunks the
            # kt-outer and first sc-outer loops need; gpsimd carries the
            # sc2+sc3 xt halves needed last
            nc.sync.dma_start(out=wq_sb[0][:], in_=wq.ap()[0])
            nc.scalar.dma_start(out=wq_sb[1][:], in_=wq.ap()[1])
            nc.sync.dma_start(out=wk_sb[0][:], in_=wk.ap()[0])
            nc.scalar.dma_start(out=wk_sb[1][:], in_=wk.ap()[1])
            for kt in range(16):
                eng = nc.sync if kt % 2 == 0 else nc.scalar
                eng.dma_start(
                    out=xts0[kt][:, 0:512],
                    in_=xt.ap()[0, kt * 128:(kt + 1) * 128, 0:512])
            # wv is first consumed only after the 16 q/k kt-steps
            nc.scalar.dma_start(out=wv_sb[:], in_=wv.ap()[:])
            for kt in range(16):
                eng = nc.sync if kt % 2 == 0 else nc.scalar
                eng.dma_start(
                    out=xts0[kt][:, 512:1024],
                    in_=xt.ap()[0, kt * 128:(kt + 1) * 128, 512:1024])
            for kt in range(16):
                nc.gpsimd.dma_start(
                    out=xts0[kt][:, 1024:2048],
                    in_=xt.ap()[0, kt * 128:(kt + 1) * 128, 1024:2048])
            nc.sync.dma_start(out=wo_t[0][:], in_=wo.ap()[0])
            nc.scalar.dma_start(out=wo_t[1][:], in_=wo.ap()[1])

            pending_tail = []  # deferred trailing stages of the previous batch

            for b in range(B):
                if b == 0:
                    xts = xts0
                else:
                    # prefetched on the gpsimd queue during batch 0's
                    # attention; WAR deps gate each tile on batch 0's QKV
                    xts = [xtp.tile([128, S], bf16, tag=f"xt{kt}",
                                    name=f"xt_1_{kt}")
                           for kt in range(16)]
                    for kt in range(16):
                        nc.gpsimd.dma_start(
                            out=xts[kt][:],
                            in_=xt.ap()[b, kt * 128:(kt + 1) * 128, :])

                q_sb = [qkvp.tile([128, S], bf16, tag=f"q{h}", name=f"q_{b}_{h}") for h in range(HPC)]
                k_sb = [qkvp.tile([128, S], bf16, tag=f"k{h}", name=f"k_{b}_{h}") for h in range(HPC)]

                # ---- QKV projection ----
                # first s-chunk: q/k kt-outer, consuming each xt piece as it
                # arrives from HBM instead of waiting for all 16.  Each head
                # packs its q (bank A) and k (bank B) groups in one psT tile.
                ssl = slice(0, 512)
                psqk = [psTp.tile([128, 1024], f32, tag="psT",
                                  name=f"psqk0_{b}_{h}") for h in range(HPC)]
                for kt in range(16):
                    for h in range(HPC):
                        nc.tensor.matmul(
                            psqk[h][:, 0:512], lhsT=wq_sb[h][:, kt, :],
                            rhs=xts[kt][:, ssl],
                            start=(kt == 0), stop=(kt == 15))
                        nc.tensor.matmul(
                            psqk[h][:, 512:1024], lhsT=wk_sb[h][:, kt, :],
                            rhs=xts[kt][:, ssl],
                            start=(kt == 0), stop=(kt == 15))
                for h in range(HPC):
                    nc.vector.tensor_scalar_add(q_sb[h][:, ssl],
                                                psqk[h][:, 0:512], bq_t[h][:])
                    nc.vector.tensor_scalar_add(k_sb[h][:, ssl],
                                                psqk[h][:, 512:1024], bk_t[h][:])
                # V st-outer (xt sc0 has fully landed by now); the four
                # 256-wide groups in one two-bank tile run sequentially —
                # two concurrently-open accumulation groups in one PSUM
                # bank corrupt the first term
                psv = psTp.tile([128, 1024], f32, tag="psT",
                                name=f"psv0_{b}")
                for st in range(4):
                    vsl = slice(st * 256, st * 256 + HPC * DH)
                    for kt in range(16):
                        nc.tensor.matmul(
                            psv[:, vsl],
                            lhsT=xts[kt][:, st * 128:(st + 1) * 128],
                            rhs=wv_sb[:, kt, :],
                            start=(kt == 0), stop=(kt == 15))
                    nc.vector.tensor_copy(v_sb[:, st, :, 0:DH], psv[:, vsl])
                # previous batch's trailing transpose/out-proj: the PSUM
                # banks are free again here, and these matmuls fill the
                # batch-boundary pipeline drain
                for fn in pending_tail:
                    fn()
                pending_tail = []
                for sc4 in range(1, 4):
                    ssl = slice(sc4 * 512, (sc4 + 1) * 512)
                    for h in range(HPC):
                        ps = psTp.tile([128, 1024], f32, tag="psT")
                        for kt in range(16):
                            nc.tensor.matmul(
                                ps[:, 0:512], lhsT=wq_sb[h][:, kt, :],
                                rhs=xts[kt][:, ssl],
                                start=(kt == 0), stop=(kt == 15))
                            nc.tensor.matmul(
                                ps[:, 512:1024], lhsT=wk_sb[h][:, kt, :],
                                rhs=xts[kt][:, ssl],
                                start=(kt == 0), stop=(kt == 15))
                        nc.vector.tensor_scalar_add(q_sb[h][:, ssl],
                                                    ps[:, 0:512], bq_t[h][:])
                        nc.vector.tensor_scalar_add(k_sb[h][:, ssl],
                                                    ps[:, 512:1024], bk_t[h][:])
                    for st4 in range(4):
                        st = sc4 * 4 + st4
                        tsl = slice(st * 128, (st + 1) * 128)
                        psv = psTp.tile([128, 1024], f32, tag="psT")
                        for kt in range(16):
                            nc.tensor.matmul(
                                psv[:, :HPC * DH], lhsT=xts[kt][:, tsl],
                                rhs=wv_sb[:, kt, :],
                                start=(kt == 0), stop=(kt == 15))
                        nc.vector.tensor_copy(v_sb[:, st, :, 0:DH],
                                              psv[:, :HPC * DH])

                # ---- attention ----
                def stage_scores(si, b=b, q_sb=q_sb, k_sb=k_sb):
                    """Transposed scores + exp for both heads of q-tile si.

                    Each two-bank PSUM chunk holds up to 4 j-blocks x 2
                    heads in [j, h, q] column order, so one ScalarE exp
                    covers the whole chunk.  Only the diagonal block gets
                    the causal mask; staircase blocks beyond the diagonal
                    are never consumed."""
                    nj = si + 1 if causal else NT
                    eT = expp.tile([128, nj, HPC, 128], bf16, tag="expT",
                                   name=f"expT_{b}_{si}")
                    for c4 in range(0, nj, 4):
                        jn = min(4, nj - c4)
                        psT = psTp.tile([128, 4, HPC, 128], f32, tag="psT",
                                        name=f"psT_{b}_{si}_{c4}")
                        for j4 in range(jn):
                            j = c4 + j4
                            for h in range(HPC):
                                nc.tensor.matmul(
                                    psT[:, j4, h, :],
                                    lhsT=k_sb[h][:, j * 128:(j + 1) * 128],
                                    rhs=q_sb[h][:, si * 128:(si + 1) * 128],
                                    start=True, stop=True)
                        if not causal:
                            mt = mbp.tile([128, 512], f32, tag="mb",
                                          name=f"mb_{b}_{si}_{c4}")
                            nc.sync.dma_start(
                                out=mt[:, :jn * 128],
                                in_=mb.ap()[c4 * 128:(c4 + jn) * 128,
                                            si * 128:(si + 1) * 128]
                                .rearrange("(j t) q -> t (j q)", t=128))
                            for h in range(HPC):
                                nc.vector.tensor_add(
                                    psT[:, :jn, h, :], psT[:, :jn, h, :],
                                    mt[:, :jn * 128])
                        nc.scalar.activation(
                            eT[:, c4:c4 + jn, :, :],
                            psT[:, :jn, :, :], AF.Exp, scale=SCALE)
                    if causal:
                        # zero the exp'd upper triangle of the diagonal block
                        # (identical to masking scores with -inf before exp)
                        for h in range(HPC):
                            nc.gpsimd.tensor_mul(eT[:, nj - 1, h, :],
                                                 eT[:, nj - 1, h, :], tri01[:])
                    return eT

                def stage_attnv(si, eT, b=b):
                    """attn@V with the ones-augmented V: PSUM [q, DH+1] where
                    column DH is the softmax denominator. Normalization =
                    VectorE reciprocal + per-partition tensor_scalar_mul."""
                    nj = si + 1 if causal else NT
                    po = pop.tile([128, HPC, DH + 4], f32, tag="po",
                                  name=f"po_{b}_{si}")  # [:, h, DH] = denom
                    o_h = []
                    for h in range(HPC):
                        for j in range(nj):
                            nc.tensor.matmul(
                                po[:, h, 0:DH + 1],
                                lhsT=eT[:, j, h, :],
                                rhs=v_sb[:, j, h, :],
                                start=(j == 0), stop=(j == nj - 1))
                        rinv = accp.tile([128, 1], f32, tag=f"rinv{h}",
                                         name=f"rinv_{b}_{si}_{h}")
                        nc.vector.reciprocal(rinv[:], po[:, h, DH:DH + 1])
                        o_sb = osbp.tile([128, DH], bf16, tag=f"osb{h}",
                                         name=f"osb_{b}_{si}_{h}")
                        nc.vector.tensor_scalar_mul(o_sb[:], po[:, h, 0:DH],
                                                    rinv[:])
                        o_h.append(o_sb)
                    return o_h

                def stage_transp(si, o_h, b=b):
                    """[q,dh]->[dh,q] PE transpose per head; bf16 eviction on
                    the gpsimd engine (scalar and vector are both loaded)."""
                    ptr = Ap.tile([128, 512], f32, tag="A",
                                  name=f"ptr_{b}_{si}")
                    for h in range(HPC):
                        nc.tensor.matmul(
                            ptr[:, h * DH:(h + 1) * DH],
                            lhsT=o_h[h][:], rhs=ident[:],
                            start=True, stop=True)
                    oT = outTp.tile([128, HPC * DH], bf16, tag="outT",
                                    name=f"oT_{b}_{si}")
                    nc.vector.tensor_copy(oT[:], ptr[:, :HPC * DH])
                    return oT

                def stage_oproj(si, oT, b=b):
                    """Out-projection with both heads in one PSUM group.
                    Output stores alternate between the sync and gpsimd
                    queues so neither carries the whole output stream."""
                    oeng = nc.sync if si % 2 == 1 else nc.gpsimd
                    ostage = outsp.tile([128, DM], bf16, tag="ostage",
                                        name=f"ostage_{b}_{si}")
                    for ncn in range(4):
                        nsl = slice(ncn * 512, (ncn + 1) * 512)
                        pso = Ap.tile([128, 512], f32, tag="A",
                                      name=f"pso_{b}_{si}_{ncn}")
                        for h in range(HPC):
                            nc.tensor.matmul(pso[:],
                                             lhsT=oT[:, h * DH:(h + 1) * DH],
                                             rhs=wo_t[h][:, nsl],
                                             start=(h == 0), stop=(h == HPC - 1))
                        if ncn % 2 == 0:
                            nc.scalar.activation(ostage[:, nsl], pso[:],
                                                 AF.Copy)
                        else:
                            nc.vector.tensor_copy(ostage[:, nsl], pso[:])
                        last_tile = (si == NT - 1)
                        if last_tile:
                            # fine-grained stores so the tail drains early
                            oeng.dma_start(
                                out=outp.ap()[b, si * 128:(si + 1) * 128, nsl],
                                in_=ostage[:, nsl])
                        elif ncn % 2 == 1:
                            oeng.dma_start(
                                out=outp.ap()[b, si * 128:(si + 1) * 128,
                                              (ncn - 1) * 512:(ncn + 1) * 512],
                                in_=ostage[:, (ncn - 1) * 512:(ncn + 1) * 512])

                # software pipeline: scores two tiles ahead; transpose one
                # and out-proj two tiles behind attn@V
                exp_q = {0: stage_scores(0), 1: stage_scores(1)}
                o_box, oT_box = {}, {}
                for si in range(NT):
                    if si + 2 < NT:
                        exp_q[si + 2] = stage_scores(si + 2)
                    o_box[si] = stage_attnv(si, exp_q.pop(si))
                    if si >= 1:
                        oT_box[si - 1] = stage_transp(si - 1, o_box.pop(si - 1))
                    if si >= 2:
                        stage_oproj(si - 2, oT_box.pop(si - 2))
                pending_tail = [
                    (lambda si=NT - 1, oh=o_box.pop(NT - 1):
                     oT_box.__setitem__(si, stage_transp(si, oh))),
                    (lambda si=NT - 2: stage_oproj(si, oT_box.pop(si))),
                    (lambda si=NT - 1: stage_oproj(si, oT_box.pop(si))),
                ]

            # final batch's trailing stages
            for fn in pending_tail:
                fn()

    nc.compile()
    return nc


def _get(causal: bool):
    if causal not in _BUILT:
        _BUILT[causal] = _build(causal)
    return _BUILT[causal]


def _rot(fr, fi, m):
    """Apply the reference's per-head rotary as a linear map on rows of m."""
    top, bot = m[:DH // 2], m[DH // 2:]
    return np.concatenate([fr[:, None] * top - fi[:, None] * bot,
                           fi[:, None] * top + fr[:, None] * bot], axis=0)


def _pack(w):
    """[DM, C] -> [128, 16*C] with [p, kt*C+c] = w[kt*128+p, c]."""
    c = w.shape[1]
    return np.ascontiguousarray(
        w.reshape(16, 128, c).transpose(1, 0, 2).reshape(128, 16 * c))


def kernel(x, w_qkv, b_qkv, w_out, b_out, fc_real, fc_imag, mask):
    x = np.asarray(x, np.float32)
    w_qkv = np.asarray(w_qkv, np.float32)
    b_qkv = np.asarray(b_qkv, np.float32)
    w_out = np.asarray(w_out, np.float32)
    b_out = np.asarray(b_out, np.float32)
    fc_real = np.asarray(fc_real, np.float32)
    fc_imag = np.asarray(fc_imag, np.float32)
    mask_np = np.asarray(mask)[0, 0]

    causal = bool(np.array_equal(
        mask_np, np.triu(np.ones((S, S), bool), 1)))

    bf = ml_dtypes.bfloat16
    xt_host = np.ascontiguousarray(x.transpose(0, 2, 1)).astype(bf)

    in_maps = []
    maskb = None
    if not causal:
        # transposed ([t, q]) additive mask for the transposed-scores layout
        maskb = np.ascontiguousarray(
            np.where(mask_np, np.float32(-1e30), np.float32(0.0)).T)
    for c in range(NCORES):
        wq_h, wk_h, bq_h, bk_h, wv_h, wo_h = [], [], [], [], [], []
        for hh in range(HPC):
            g = c * HPC + hh
            fr = fc_real[0, g, :]
            fi = fc_imag[0, g, :]
            wq_h.append(_pack(
                _rot(fr, fi, w_qkv[g * DH:(g + 1) * DH, :]).T).astype(bf))
            wk_h.append(_pack(
                _rot(fr, fi, w_qkv[DM + g * DH:DM + (g + 1) * DH, :]).T
            ).astype(bf))
            bq_h.append(_rot(fr, fi, b_qkv[g * DH:(g + 1) * DH, None])[:, 0])
            bk_h.append(_rot(fr, fi,
                             b_qkv[DM + g * DH:DM + (g + 1) * DH, None])[:, 0])
            wv_h.append(w_qkv[2 * DM + g * DH:2 * DM + (g + 1) * DH, :].T)
            wo_h.append(np.ascontiguousarray(
                w_out[:, g * DH:(g + 1) * DH].T).astype(bf))
        m = {
            "xt": xt_host,
            "wq": np.stack(wq_h),
            "wk": np.stack(wk_h),
            "wv": _pack(np.concatenate(wv_h, axis=1)).astype(bf),
            "bq": np.stack(bq_h).astype(np.float32),
            "bk": np.stack(bk_h).astype(np.float32),
            "wo": np.stack(wo_h),
        }
        if not causal:
            m["maskb"] = maskb
        in_maps.append(m)

    from concourse.bass_utils import run_bass_kernel_spmd
    nc = _get(causal)
    global _LAST_IN_MAPS
    _LAST_IN_MAPS = in_maps
    trace = os.environ.get("MHA_TRACE") == "1"
    res = run_bass_kernel_spmd(nc, in_maps, core_ids=list(range(NCORES)),
                               trace=trace)
    if trace:
        kernel.last_results = res

    out = res.results[0]["outp"].astype(np.float64)
    for c in range(1, NCORES):
        out += res.results[c]["outp"]
    b_v = b_qkv[2 * DM:]
    out += (b_out + w_out @ b_v)[None, None, :]
    return out.astype(np.float32)
